# revision 60
# baseline (speedup 1.0000x reference)
"""MinimalMambaBlock Trainium2 kernel — fp8 (e4m3) DoubleRow matmul version.

Sharding: 8 cores = 4 batch rows x 2 sequence halves. Each core processes
T = 32 + 1024 tokens of one batch row with the 32-token halo at the FRONT:
second-half cores warm the linear recurrence up through real tokens; first
half cores get 32 zero rows plus a per-core scan-mask column (m0) that
zeroes the recurrence carry exactly at the true sequence start. Every core
outputs the uniform window [32:1056).

All five projections run as fp8e4 (e4m3) matmuls in DoubleRow perf mode
(256-deep contraction per pass, 2x the fp32r MAC rate). PSUM accumulates in
fp32. Per-tensor power-of-2 scales keep operands inside e4m3 range (max 240):
  xn*16, u*32, y*64, in/gate weights *2048, b/c/d/out weights *4096.
Scale corrections fold into the existing bias/activation steps. The h scan
carries an extra beta = s_y/(s_wc*s_u) factor so phase D's
(ps_c + c_b') * h' fuses into one scalar_tensor_tensor with no extra scaling.

Device pipeline (activations in [channel, time] layout after the norm):
  A: load x [t,d] -> RMSNorm (*s_x, bf16) -> PE-transpose -> fp8 xp [d,2,t]
  B: u = (in_w @ xn + in_b) * sigmoid(gate_w @ xn + gate_b) -> fp8 u pairs
     (drains split DVE/ACT, multiplies deferred one iteration and split
      DVE/Pool so psum-slot releases never queue behind cross-engine waits)
  C: b = b_w @ u + b_b -> h' = tensor_tensor_scan(a*m0, b*beta)
  D: h' *= (c_w @ u + c_b')            (stt, in place)
  E: y = h' + (d_w @ u + d_b)*s_y      -> fp8 y pairs
  F: out[t,d] = y^T @ out_w (y stationary, token-major: no transposes)
     + (x + out_b) via one in-place stt per block; one store per token tile

Perf notes (measured on hw): 256-col psum blocks are the sweet spot (512
compiles+passes but loses PE pipeline overlap); LDWEIGHTS mostly pipelines
under matmuls so weight-major reordering is not worth it; Pool engine ops
are ~2.5x slower than DVE and cannot read PSUM.
"""

import os
import sys
from contextlib import ExitStack

import numpy as np
import ml_dtypes

sys.path.insert(0, "/opt/trn_rl_repo")

import concourse.bass as bass
import concourse.mybir as mybir
import concourse.tile as tile
from concourse.bass_utils import run_bass_kernel_spmd
from concourse.masks import make_identity

F32 = mybir.dt.float32
BF16 = mybir.dt.bfloat16
FP8 = mybir.dt.float8e4
E4M3 = ml_dtypes.float8_e4m3
AF = mybir.ActivationFunctionType
OP = mybir.AluOpType
DR = mybir.MatmulPerfMode.DoubleRow

DIM = 1024
INNER = 2048
B = 4
S = 2048
EPS = 1e-6
HALO = 32
T = 1024 + HALO  # 1056
NKD = DIM // 128  # 8 d-tiles
NKI = INNER // 128  # 16 inner tiles
KPD = NKD // 2  # 4 k-pairs over model dim
KPI = NKI // 2  # 8 k-pairs over inner dim
# Halo layout: every core's 32 halo tokens sit at the FRONT (cols 0:32).
# Second-half cores warm the recurrence up through them; first-half cores
# get 32 zero-padded rows plus a scan-reset mask column at t=32, so the
# recurrence restarts exactly at the true sequence start. The output
# window is uniformly tokens [32:1056).
# token tiles for transpose/norm (partition dim = tokens)
TTILES = [(i * 128, 128) for i in range(8)] + [(1024, HALO)]
# free-dim blocks for B/C matmuls (256-col blocks: wider 512-col blocks
# compile and pass but lose PE pipeline overlap on hw — measured slower)
TBLOCKS = [(0, 256), (256, 256), (512, 256), (768, 256), (1024, T - 1024)]
# output-window blocks for D/E/F (4 clean 256-blocks)
OBLOCKS = [(HALO, 256), (HALO + 256, 256), (HALO + 512, 256), (HALO + 768, 256)]
# output-window token tiles for phase F
FTILES = [(HALO + i * 128, 128) for i in range(8)]

# power-of-2 operand scales (validated against e4m3 max 240 on the fixed
# seed-0 inputs: scaled maxima are 87/72/60; weight bounds are exact
# 1/sqrt(fan_in) so weight maxima are static)
S_X = 16.0
S_U = 32.0
S_Y = 64.0
S_WI = 2048.0
S_WG = 2048.0
S_WB = 4096.0
S_WC = 4096.0
S_WD = 4096.0
S_WO = 4096.0
BETA = S_Y / (S_WC * S_U)  # extra scale carried by h'

_CACHED = {}


def build_nc():
    nc = bass.Bass("TRN2")

    # x is passed from the host as bf16: halves the descriptor-rate-bound
    # load time of phases A and F; the residual add keeps f32 accumulation
    x = nc.dram_tensor("x", [T, DIM], BF16, kind="ExternalInput")
    # DoubleRow weight strips, pre-laid-out host side (see _prep_shared):
    # w_ig[p, mi, half, j, i, m]; others w[p, mt, j, i, m]
    w_ig = nc.dram_tensor("w_ig", [128, NKI * 2 * KPD * 2 * 128], FP8,
                          kind="ExternalInput")
    w_b = nc.dram_tensor("w_b", [128, NKI * KPI * 2 * 128], FP8,
                         kind="ExternalInput")
    w_c = nc.dram_tensor("w_c", [128, NKI * KPI * 2 * 128], FP8,
                         kind="ExternalInput")
    w_d = nc.dram_tensor("w_d", [128, NKI * KPI * 2 * 128], FP8,
                         kind="ExternalInput")
    w_om = nc.dram_tensor("w_om", [128, KPI * 2 * DIM], FP8,
                          kind="ExternalInput")
    # per-channel vectors pre-laid-out host-side as [128, n_tiles]
    bias_ig = nc.dram_tensor("bias_ig", [128, 2 * NKI], F32, kind="ExternalInput")
    bias_bcd = nc.dram_tensor("bias_bcd", [128, 3 * NKI], F32, kind="ExternalInput")
    bias_ob = nc.dram_tensor("bias_ob", [128, DIM], F32, kind="ExternalInput")
    a_in = nc.dram_tensor("a_in", [128, NKI], F32, kind="ExternalInput")
    # per-core scan mask for block 0: ones, except first-half cores carry a
    # zero at column HALO which resets the recurrence at the true seq start
    m0_in = nc.dram_tensor("m0", [128, 256], F32, kind="ExternalInput")
    out = nc.dram_tensor("out", [T, DIM], F32, kind="ExternalOutput")

    w_ig_r = w_ig.ap().rearrange("p (mi h j i m) -> p mi h j i m",
                                 mi=NKI, h=2, j=KPD, i=2)
    w_b_r = w_b.ap().rearrange("p (mt j i m) -> p mt j i m", mt=NKI, j=KPI, i=2)
    w_c_r = w_c.ap().rearrange("p (mt j i m) -> p mt j i m", mt=NKI, j=KPI, i=2)
    w_d_r = w_d.ap().rearrange("p (mt j i m) -> p mt j i m", mt=NKI, j=KPI, i=2)
    w_om_r = w_om.ap().rearrange("p (j i d) -> p j i d", j=KPI, i=2)
    x_ap = x.ap()
    out_ap = out.ap()

    with tile.TileContext(nc) as tc, ExitStack() as ctx:
        statics = ctx.enter_context(tc.tile_pool(name="statics", bufs=1))
        xwork = ctx.enter_context(tc.tile_pool(name="xwork", bufs=4))
        wpool = ctx.enter_context(tc.tile_pool(name="wpool", bufs=4))
        work = ctx.enter_context(tc.tile_pool(name="work", bufs=3))
        small = ctx.enter_context(tc.tile_pool(name="small", bufs=8))
        frow = ctx.enter_context(tc.tile_pool(name="frow", bufs=4))
        psA = ctx.enter_context(tc.tile_pool(name="psA", bufs=1, space="PSUM"))

        identF = statics.tile([128, 128], BF16, tag="identF")
        make_identity(nc, identF)
        eps_t = statics.tile([128, 1], F32, tag="eps_t")
        nc.vector.memset(eps_t, EPS / (S_X * S_X))


        # persistent activations
        xp = [statics.tile([128, 2, T], FP8, tag=f"xp{j}", name=f"xp{j}")
              for j in range(KPD)]
        up = [statics.tile([128, 2, T], FP8, tag=f"up{j}", name=f"up{j}")
              for j in range(KPI)]
        yp = [statics.tile([128, 2, T], FP8, tag=f"yp{j}", name=f"yp{j}")
              for j in range(KPI)]
        # h/b_full/a_bc in bf16: DVE runs 16-bit at 2x, and the scan's
        # internal state stays fp32 regardless of operand dtype, so only
        # the stored h rounds (~0.2%)
        h = [statics.tile([128, T], BF16, tag=f"h{i}", name=f"h{i}")
             for i in range(NKI)]

        # ---- Phase A: load + RMSNorm (*S_X) + fp8 + transpose -> xp ----
        # x loads stay on the sync queue: spreading them across the
        # scalar/gpsimd queues was measured much slower (DMA issues block
        # those engines' compute work behind them)
        for tt, (t0, tl) in enumerate(TTILES):
            x_t = xwork.tile([128, DIM], BF16, tag="x_t")
            nc.sync.dma_start(out=x_t[:tl, :], in_=x_ap[t0 : t0 + tl, :])
            sq_t = xwork.tile([128, DIM], F32, tag="sq_t")
            sumsq = small.tile([128, 1], F32, tag="sumsq")
            nc.scalar.activation(
                sq_t[:tl, :], x_t[:tl, :], AF.Square, accum_out=sumsq[:tl, :]
            )
            rms = small.tile([128, 1], F32, tag="rms")
            # rms = sqrt(mean + eps) / S_X
            nc.scalar.activation(
                rms[:tl, :], sumsq[:tl, :], AF.Sqrt, bias=eps_t[:tl, :],
                scale=1.0 / (DIM * S_X * S_X),
            )
            scale = small.tile([128, 1], F32, tag="scale")
            nc.vector.reciprocal(scale[:tl, :], rms[:tl, :])
            xn_t = xwork.tile([128, DIM], BF16, tag="xn_t")
            nc.vector.tensor_scalar_mul(xn_t[:tl, :], x_t[:tl, :], scale[:tl, :])
            for di in range(NKD):
                tr = psA.tile([128, 128], BF16, tag="tr", bufs=2, name="tr_a")
                nc.tensor.transpose(
                    tr[:, :tl], xn_t[:tl, di * 128 : (di + 1) * 128],
                    identF[:tl, :tl],
                )
                # split the fp8 casts across DVE and ACT so neither engine
                # backs up while phase B's drains start to overlap phase A
                if di % 2 == 0:
                    nc.vector.tensor_copy(
                        xp[di // 2][:, di % 2, t0 : t0 + tl], tr[:, :tl]
                    )
                else:
                    nc.scalar.copy(
                        xp[di // 2][:, di % 2, t0 : t0 + tl], tr[:, :tl]
                    )

        # static per-channel vectors (emitted after phase A so the x-tile DMAs
        # lead the queue and the first transposes start sooner)
        b_ig = statics.tile([128, 2 * NKI], F32, tag="b_ig")
        nc.sync.dma_start(out=b_ig, in_=bias_ig.ap())
        b_bcd = statics.tile([128, 3 * NKI], F32, tag="b_bcd")
        nc.sync.dma_start(out=b_bcd, in_=bias_bcd.ap())
        a_t = statics.tile([128, NKI], F32, tag="a_t")
        nc.sync.dma_start(out=a_t, in_=a_in.ap())
        m0_t = statics.tile([128, 256], F32, tag="m0_t")
        nc.sync.dma_start(out=m0_t, in_=m0_in.ap())

        # ---- Phase B: u = (in @ xn + in_b) * sigmoid(gate @ xn + gate_b) ----
        # The u = u32*g multiplies are deferred by one iteration: when they
        # are emitted in program order their operands are already complete,
        # so the DVE/Pool queues never stall on a sigmoid while the next
        # iteration's psum drains sit behind them.
        def flush_mult(pmi, pu32, pgs):
            for bi, (n0, nl) in enumerate(TBLOCKS):
                eng = nc.vector if bi < 3 else nc.gpsimd
                eng.tensor_mul(
                    up[pmi // 2][:, pmi % 2, n0 : n0 + nl],
                    pu32[:, n0 : n0 + nl], pgs[bi][:, :nl],
                )

        pending = None
        for mi in range(NKI):
            w_s = wpool.tile([128, 2, KPD, 2, 128], FP8, tag="ws", name="w_ig_s")
            nc.sync.dma_start(out=w_s, in_=w_ig_r[:, mi])
            ps_us = [psA.tile([128, nl], F32, tag=("ps" if bi < 4 else "tr"),
                              bufs=(6 if bi < 4 else 2), name=f"ps_u{bi}")
                     for bi, (n0, nl) in enumerate(TBLOCKS)]
            for j in range(KPD):
                for bi, (n0, nl) in enumerate(TBLOCKS):
                    nc.tensor.matmul(
                        ps_us[bi], w_s[:, 0, j], xp[j][:, :, n0 : n0 + nl],
                        start=(j == 0), stop=(j == KPD - 1), perf_mode=DR,
                    )
            u32 = work.tile([128, T], F32, tag="fullT", name="u32")
            for bi, (n0, nl) in enumerate(TBLOCKS):
                # drains gate the gate-matmuls' psum slots; split DVE/ACT so
                # neither engine serializes the pipeline (gpsimd can't read
                # PSUM)
                if bi < 3:
                    nc.vector.tensor_scalar(
                        u32[:, n0 : n0 + nl], ps_us[bi],
                        S_U / (S_WI * S_X), b_ig[:, mi : mi + 1],
                        op0=OP.mult, op1=OP.add,
                    )
                else:
                    nc.scalar.activation(
                        u32[:, n0 : n0 + nl], ps_us[bi], AF.Identity,
                        bias=b_ig[:, mi : mi + 1], scale=S_U / (S_WI * S_X),
                    )
            ps_gs = [psA.tile([128, nl], F32, tag=("ps" if bi < 4 else "tr"),
                              bufs=(6 if bi < 4 else 2), name=f"ps_g{bi}")
                     for bi, (n0, nl) in enumerate(TBLOCKS)]
            for j in range(KPD):
                for bi, (n0, nl) in enumerate(TBLOCKS):
                    nc.tensor.matmul(
                        ps_gs[bi], w_s[:, 1, j], xp[j][:, :, n0 : n0 + nl],
                        start=(j == 0), stop=(j == KPD - 1), perf_mode=DR,
                    )
            g_sbs = []
            for bi, (n0, nl) in enumerate(TBLOCKS):
                g_sb = small.tile([128, 256], F32, tag="g_sb", bufs=10)
                nc.scalar.activation(
                    g_sb[:, :nl], ps_gs[bi], AF.Sigmoid,
                    bias=b_ig[:, NKI + mi : NKI + mi + 1],
                    scale=1.0 / (S_WG * S_X),
                )
                g_sbs.append(g_sb)
            if pending is not None:
                flush_mult(*pending)
            pending = (mi, u32, g_sbs)
        flush_mult(*pending)

        # ---- Phase C: b = b_w @ u + b_b ; h' = scan(a, b*BETA) ----
        for ji in range(NKI):
            w_s = wpool.tile([128, KPI, 2, 128], FP8, tag="ws", name="w_b_s")
            nc.sync.dma_start(out=w_s, in_=w_b_r[:, ji])
            pss = [psA.tile([128, nl], F32, tag=("ps" if bi < 4 else "tr"),
                            bufs=(6 if bi < 4 else 2), name=f"ps_b{bi}")
                   for bi, (n0, nl) in enumerate(TBLOCKS)]
            for j in range(KPI):
                for bi, (n0, nl) in enumerate(TBLOCKS):
                    nc.tensor.matmul(
                        pss[bi], w_s[:, j], up[j][:, :, n0 : n0 + nl],
                        start=(j == 0), stop=(j == KPI - 1), perf_mode=DR,
                    )
            b_full = work.tile([128, T], BF16, tag="fullT", name="b_full")
            for bi, (n0, nl) in enumerate(TBLOCKS):
                nc.scalar.activation(
                    b_full[:, n0 : n0 + nl], pss[bi], AF.Identity,
                    bias=b_bcd[:, ji : ji + 1], scale=BETA / (S_WB * S_U),
                )
            a_bc = small.tile([128, 256], BF16, tag="a_bc", bufs=2)
            nc.vector.memset(a_bc, 1.0)
            nc.vector.tensor_scalar_mul(a_bc, a_bc, a_t[:, ji : ji + 1])
            # block 0 carries the per-core reset mask (m0 column HALO)
            a_bc0 = small.tile([128, 256], BF16, tag="a_bc0", bufs=2)
            nc.vector.tensor_scalar_mul(a_bc0, m0_t, a_t[:, ji : ji + 1])
            for bi, (n0, nl) in enumerate(TBLOCKS):
                init = 0.0 if bi == 0 else h[ji][:, n0 - 1 : n0]
                nc.vector.tensor_tensor_scan(
                    h[ji][:, n0 : n0 + nl],
                    (a_bc0 if bi == 0 else a_bc)[:, :nl],
                    b_full[:, n0 : n0 + nl], init, op0=OP.mult, op1=OP.add,
                )

        # ---- Phase D: h' *= (c_w @ u + c_b')   (in place) ----
        for ji in range(NKI):
            w_s = wpool.tile([128, KPI, 2, 128], FP8, tag="ws", name="w_c_s")
            nc.sync.dma_start(out=w_s, in_=w_c_r[:, ji])
            pss = [psA.tile([128, nl], F32, tag="ps", bufs=6, name=f"ps_c{bi}")
                   for bi, (n0, nl) in enumerate(OBLOCKS)]
            for j in range(KPI):
                for bi, (n0, nl) in enumerate(OBLOCKS):
                    nc.tensor.matmul(
                        pss[bi], w_s[:, j], up[j][:, :, n0 : n0 + nl],
                        start=(j == 0), stop=(j == KPI - 1), perf_mode=DR,
                    )
            for bi, (n0, nl) in enumerate(OBLOCKS):
                nc.vector.scalar_tensor_tensor(
                    h[ji][:, n0 : n0 + nl], pss[bi],
                    b_bcd[:, NKI + ji : NKI + ji + 1], h[ji][:, n0 : n0 + nl],
                    op0=OP.add, op1=OP.mult,
                )

        # phase-F statics, emitted late so the 2MB transfer rides under
        # phases D/E instead of competing with phase A's x loads
        w_om_t = statics.tile([128, KPI, 2, DIM], FP8, tag="w_om")
        nc.gpsimd.dma_start(out=w_om_t, in_=w_om_r)
        bias_ob_t = statics.tile([128, DIM], F32, tag="bias_ob")
        nc.gpsimd.dma_start(out=bias_ob_t, in_=bias_ob.ap())

        # ---- Phase E: y = h' + (d_w @ u + d_b)*S_Y  -> fp8 yp ----
        for ji in range(NKI):
            w_s = wpool.tile([128, KPI, 2, 128], FP8, tag="ws", name="w_d_s")
            nc.sync.dma_start(out=w_s, in_=w_d_r[:, ji])
            pss = [psA.tile([128, nl], F32, tag="ps", bufs=6, name=f"ps_d{bi}")
                   for bi, (n0, nl) in enumerate(OBLOCKS)]
            for j in range(KPI):
                for bi, (n0, nl) in enumerate(OBLOCKS):
                    nc.tensor.matmul(
                        pss[bi], w_s[:, j], up[j][:, :, n0 : n0 + nl],
                        start=(j == 0), stop=(j == KPI - 1), perf_mode=DR,
                    )
            dd32 = work.tile([128, T], F32, tag="fullT", name="dd32")
            for bi, (n0, nl) in enumerate(OBLOCKS):
                nc.scalar.activation(
                    dd32[:, n0 : n0 + nl], pss[bi], AF.Identity,
                    bias=b_bcd[:, 2 * NKI + ji : 2 * NKI + ji + 1],
                    scale=S_Y / (S_WD * S_U),
                )
            for bi, (n0, nl) in enumerate(OBLOCKS):
                nc.vector.tensor_add(
                    yp[ji // 2][:, ji % 2, n0 : n0 + nl],
                    h[ji][:, n0 : n0 + nl], dd32[:, n0 : n0 + nl],
                )

        # ---- Phase F: out[t, d] = y^T @ out_w + out_b + x  (token-major;
        # y is the matmul stationary so no transposes are needed, the bias
        # folds into the residual row, and each token tile streams its own
        # store as soon as it completes) ----
        for tt, (t0, tl) in enumerate(FTILES):
            x_r = statics.tile([128, DIM], BF16, tag=f"h{tt}",
                               name=f"x_row{tt}")
            (nc.gpsimd if tt % 2 else nc.sync).dma_start(
                out=x_r[:tl, :], in_=x_ap[t0 : t0 + tl, :])
            xb = statics.tile([128, DIM], F32, tag=f"h{8 + tt}", name=f"xb{tt}")
            nc.vector.tensor_add(xb, x_r, bias_ob_t)
            pss = [psA.tile([128, 256], F32, tag="ps", bufs=6, name=f"ps_o{bi}")
                   for bi in range(4)]
            for j in range(KPI):
                for bi in range(4):
                    nc.tensor.matmul(
                        pss[bi], yp[j][:, :, t0 : t0 + tl],
                        w_om_t[:, j, :, bi * 256 : (bi + 1) * 256],
                        start=(j == 0), stop=(j == KPI - 1), perf_mode=DR,
                    )
            for bi in range(4):
                nc.vector.scalar_tensor_tensor(
                    xb[:, bi * 256 : (bi + 1) * 256], pss[bi],
                    1.0 / (S_WO * S_Y), xb[:, bi * 256 : (bi + 1) * 256],
                    op0=OP.mult, op1=OP.add,
                )
            (nc.gpsimd if tt % 2 else nc.sync).dma_start(
                out=out_ap[t0 : t0 + tl, :], in_=xb[:tl, :])

    # walrus in this container only encodes 1 sync-wait on CTRL instructions
    from birfix_embed import patch_nc

    patch_nc(nc)
    return nc


# ---- embedded birfix (kernel.py must be self-contained) ----
def _enable_ldw_opt():
    """Flip walrus --enable-ldw-opt so consecutive same-weight matmuls skip
    the redundant LDWEIGHTS reload."""
    from concourse import bass_utils as _bu

    if getattr(_bu, "_ldw_opt_patched", False):
        return
    _orig = _bu.run_command

    def patched(argv, **kw):
        argv = ["--enable-ldw-opt=true" if a == "--enable-ldw-opt=false" else a
                for a in argv]
        return _orig(argv, **kw)

    _bu.run_command = patched
    _bu._ldw_opt_patched = True


# NOTE: not enabled — the Tile legalizer splits fp8 DoubleRow matmuls into
# explicit Ldweights+Matmult, and walrus rejects standalone Ldweights when
# --enable-ldw-opt=true. Ldweights dedup happens at legalize time instead.
# _enable_ldw_opt()


def _install_birfix():
    import json as _json
    import types

    mod = types.ModuleType("birfix_embed")

    CTRL = {"Drain", "NoOp", "EventSemaphore", "TriggeredCopy", "RegisterMove",
            "UnconditionalBranch", "Halt"}
    MAX_COMPUTE_WAITS = 1

    def dedup_ldweights(d):
        """Drop Ldweights whose stationary operand is already loaded.

        The Tile legalizer emits one Ldweights per (DoubleRow) Matmult; the
        PE array keeps its stationary across matmuls, so within a run of
        same-weight matmuls only the first load is needed. Any transpose or
        self-loading Matmult clobbers the array and resets tracking. The BIR
        here is post-schedule, so per-engine order is final."""
        removed = 0
        for fn in d.get("functions", []):
            for bb in fn.get("blocks", fn.get("basicblocks", [])):
                insts = bb.get("instructions", [])
                out = []
                loaded = None
                for inst in insts:
                    if inst.get("engine") != "PE":
                        out.append(inst)
                        continue
                    op = inst.get("opcode")
                    if op == "Ldweights":
                        sig = _json.dumps(
                            [inst.get("ins"), inst.get("perf_mode"),
                             inst.get("tile_position"), inst.get("tile_size"),
                             inst.get("is_transpose")],
                            sort_keys=True,
                        )
                        sync = inst.get("sync_info") or {}
                        if sig == loaded and not sync.get("on_update"):
                            waits = sync.get("on_wait") or []
                            if waits:
                                out.append({
                                    "engine": "PE", "ins": [],
                                    "name": inst["name"] + "_dd",
                                    "opcode": "NoOp", "outs": [],
                                    "sync_info": {"on_update": [],
                                                  "on_wait": waits},
                                })
                            removed += 1
                            continue
                        loaded = sig
                        out.append(inst)
                    elif op == "Matmult":
                        if inst.get("is_transpose") or inst.get("ldweights", True):
                            loaded = None
                        out.append(inst)
                    else:
                        out.append(inst)
                bb["instructions"] = out
        return removed

    def fix_bir_json(bir, max_ctrl=1, max_compute=MAX_COMPUTE_WAITS):
        d = _json.loads(bir)
        n_removed = dedup_ldweights(d)
        sys.stderr.write(f"birfix: removed {n_removed} redundant Ldweights\n")
        n_split = 0
        for fn in d.get("functions", []):
            for bb in fn.get("blocks", fn.get("basicblocks", [])):
                insts = bb.get("instructions", [])
                out = []
                changed = False
                for inst in insts:
                    sync = inst.get("sync_info")
                    cap = max_ctrl if inst.get("opcode") in CTRL else max_compute
                    if sync and len(sync.get("on_wait") or []) > cap:
                        waits = sync["on_wait"]
                        keep = waits[-cap:]
                        extra = waits[:-cap]
                        for i in range(0, len(extra), max_ctrl):
                            out.append(
                                {
                                    "engine": inst["engine"],
                                    "ins": [],
                                    "name": inst["name"] + f"_ws{i}",
                                    "opcode": "NoOp",
                                    "outs": [],
                                    "sync_info": {
                                        "on_update": [],
                                        "on_wait": extra[i : i + max_ctrl],
                                    },
                                }
                            )
                            n_split += 1
                        sync["on_wait"] = keep
                        changed = True
                    out.append(inst)
                if changed:
                    bb["instructions"] = out
        return _json.dumps(d).encode(), n_split

    def patch_nc(nc, max_ctrl=1, max_compute=MAX_COMPUTE_WAITS):
        orig = nc.to_json_bytes

        def patched():
            fixed, _ = fix_bir_json(orig(), max_ctrl, max_compute)
            return fixed

        nc.to_json_bytes = patched
        return nc

    mod.fix_bir_json = fix_bir_json
    mod.patch_nc = patch_nc
    sys.modules["birfix_embed"] = mod


_install_birfix()


def _install_ntff_hook():
    """The image lacks antenv.axon_hooks; recreate it so trace=True works."""
    import types

    if "antenv.axon_hooks" in sys.modules:
        return
    try:
        from trn_agent_boot.trn_boot import _ntff_profile_via_ctypes

        hook = _ntff_profile_via_ctypes("/opt/axon/libaxon_pjrt.so")
    except Exception:
        hook = None
    mod = types.ModuleType("antenv.axon_hooks")
    mod.get_axon_ntff_profile_hook = lambda: hook
    mod.set_axon_ntff_profile_hook = lambda h: None
    sys.modules["antenv.axon_hooks"] = mod


# ---- two-pass build: capture schedule manifest, reorder matmuls to
# weight-major (dependency- and slot-safe), rebuild with the manifest ----
def _fishpath_compat():
    from concourse._compat import FishPath

    if not hasattr(FishPath, "open"):
        def _open(self, mode="r"):
            if "w" in mode:
                self._path.parent.mkdir(parents=True, exist_ok=True)
            return open(self._path, mode)
        FishPath.open = _open
    if not hasattr(FishPath, "makedirs"):
        FishPath.makedirs = (
            lambda self: self._path.mkdir(parents=True, exist_ok=True))
    if not hasattr(FishPath, "is_file"):
        FishPath.is_file = lambda self: self._path.is_file()
    if not hasattr(FishPath, "parent"):
        FishPath.parent = property(lambda self: FishPath(self._path.parent))
    if not hasattr(FishPath, "__fspath__"):
        FishPath.__fspath__ = lambda self: str(self._path)


def _rewrite_manifest(mdir, bir, releases):
    """Reorder the captured manifest so DoubleRow matmuls run weight-major
    (j-outer), respecting data deps and tile slot reuse, so the birfix
    Ldweights dedup can drop redundant PE weight loads."""
    import glob as _glob
    import heapq
    import json as _json
    from collections import defaultdict

    mpath = _glob.glob(os.path.join(mdir, "*.json"))[0]
    with open(mpath) as f:
        m = _json.load(f)
    dpath = _glob.glob(os.path.join(mdir, "*_debug_info/instruction_deps.json"))[0]
    with open(dpath) as f:
        deps = _json.load(f)

    meta = {}
    readers = defaultdict(set)
    writers = defaultdict(set)
    for fn in bir.get("functions", []):
        for bb in fn.get("blocks", fn.get("basicblocks", [])):
            for inst in bb.get("instructions", []):
                nm = inst.get("name")
                for a in inst.get("ins", []) or []:
                    if isinstance(a, dict) and a.get("memref"):
                        readers[a["memref"]].add(nm)
                for a in inst.get("outs", []) or []:
                    if isinstance(a, dict) and a.get("memref"):
                        writers[a["memref"]].add(nm)
                if (inst.get("opcode") == "Matmult"
                        and not inst.get("is_transpose")
                        and inst.get("perf_mode") == "DoubleRow"):
                    wap = inst["ins"][1]
                    meta[nm] = (wap["memref"], wap["offset"])
    for relname, tname in releases.items():
        readers[tname].add(relname)

    slot_groups = defaultdict(list)
    for tname, (addr, space) in m["addresses"].items():
        slot_groups[(space, addr)].append(tname)

    def alloc_id(tname):
        try:
            return int(tname.rsplit("_", 1)[1])
        except ValueError:
            return 0

    for block, order in m["order"].items():
        rank = {}
        groups = defaultdict(list)
        for pos, e in enumerate(order):
            rank[e["name"]] = pos
            if e["engine"] == "PE" and e["name"] in meta:
                groups[meta[e["name"]][0]].append(pos)
        for w, positions in groups.items():
            tagged = sorted(
                (meta[order[p]["name"]][1], i, order[p]["name"])
                for i, p in enumerate(positions)
            )
            for p, (_, _, name) in zip(positions, tagged):
                rank[name] = p
        # emit all non-PE entries (drains, DMAs, virtual releases) as soon as
        # they are ready — their original positions are stale after the
        # matmul permutation and would otherwise delay psum-slot releases,
        # breaking the same-weight matmul runs the dedup relies on
        for e in order:
            if e["engine"] != "PE":
                rank[e["name"]] = -1
        for relname in releases:
            if relname in rank:
                rank[relname] = -1

        entry_by_name = {e["name"]: e for e in order}
        succ = defaultdict(list)
        indeg = {e["name"]: 0 for e in order}
        edges = set()

        def add_edge(a, b):
            if a != b and (a, b) not in edges:
                edges.add((a, b))
                succ[a].append(b)
                indeg[b] += 1

        for name, dd in deps.items():
            if name not in indeg:
                continue
            for pred in set(dd.get("pre_data", []) + dd.get("pre_no_sync", [])):
                if pred in indeg:
                    add_edge(pred, name)
        for (space, addr), tiles in slot_groups.items():
            if len(tiles) < 2:
                continue
            tiles = sorted(tiles, key=alloc_id)
            for t1, t2 in zip(tiles, tiles[1:]):
                uses = (readers[t1] | writers[t1]) & indeg.keys()
                wrts = writers[t2] & indeg.keys()
                for u in uses:
                    for wv in wrts:
                        add_edge(u, wv)
        # chain weight-groups so a group's matmuls finish before the next
        # group starts — keeps same-weights matmuls consecutive on the PE
        # queue (the Ldweights dedup then drops ~4/5 of the weight loads)
        glist = sorted(groups.items(), key=lambda kv: min(kv[1]))
        for (w1, p1), (w2, p2) in zip(glist, glist[1:]):
            last = max(p1, key=lambda p: rank[order[p]["name"]])
            first = min(p2, key=lambda p: rank[order[p]["name"]])
            add_edge(order[last]["name"], order[first]["name"])

        heap = [(rank[nm], nm) for nm, c in indeg.items() if c == 0]
        heapq.heapify(heap)
        new_order = []
        while heap:
            _, nm = heapq.heappop(heap)
            new_order.append(entry_by_name[nm])
            for s in succ[nm]:
                indeg[s] -= 1
                if indeg[s] == 0:
                    heapq.heappush(heap, (rank[s], s))
        assert len(new_order) == len(order), (len(new_order), len(order))
        m["order"][block] = new_order

    with open(mpath, "w") as f:
        _json.dump(m, f)


def build_nc_manifest():
    import json as _json
    import shutil
    import tempfile

    _fishpath_compat()
    mdir = tempfile.mkdtemp(prefix="bass_manifest_")
    saved = {k: os.environ.get(k) for k in
             ("TILE_CAPTURE_MANIFEST_PATH", "TILE_SCHEDULER",
              "TILE_LOAD_MANIFEST_PATH")}
    try:
        os.environ["TILE_CAPTURE_MANIFEST_PATH"] = mdir
        os.environ.pop("TILE_SCHEDULER", None)
        os.environ.pop("TILE_LOAD_MANIFEST_PATH", None)
        nc1 = build_nc()
        bir = _json.loads(nc1.to_json_bytes())
        releases = {}
        for nm, inst in nc1.inst_map.items():
            if (type(inst).__name__ == "BassTileRelease"
                    and inst.bass_tile is not None):
                releases[nm] = inst.bass_tile.name
        _rewrite_manifest(mdir, bir, releases)
        del nc1, bir
        os.environ.pop("TILE_CAPTURE_MANIFEST_PATH", None)
        os.environ["TILE_SCHEDULER"] = "manifest"
        os.environ["TILE_LOAD_MANIFEST_PATH"] = mdir
        nc2 = build_nc()
        return nc2
    finally:
        for k, v in saved.items():
            if v is None:
                os.environ.pop(k, None)
            else:
                os.environ[k] = v
        shutil.rmtree(mdir, ignore_errors=True)


def _prep_dr(W, s):
    """[M, K] weight -> DoubleRow strip layout [128, MT*KP*2*128] fp8,
    where strip[p, mt, j, i, m] = (W*s)[mt*128+m, j*256+i*128+p]."""
    M, K = W.shape
    MT, KP = M // 128, K // 256
    Wq = np.clip(W.astype(np.float64) * s, -240.0, 240.0)
    arr = np.ascontiguousarray(Wq.T).reshape(KP, 2, 128, MT, 128)
    arr = np.ascontiguousarray(arr.transpose(2, 3, 0, 1, 4))  # p mt j i m
    return arr.reshape(128, MT * KP * 2 * 128).astype(E4M3)


def _prep_mov(W, s):
    """[M, K] weight -> DoubleRow moving layout [128, KP*2*M] fp8,
    where mov[p, j, i, d] = (W*s)[d, j*256+i*128+p]."""
    M, K = W.shape
    KP = K // 256
    Wq = np.clip(W.astype(np.float64) * s, -240.0, 240.0)
    arr = np.ascontiguousarray(Wq.T).reshape(KP, 2, 128, M)
    arr = np.ascontiguousarray(arr.transpose(2, 0, 1, 3))  # p j i d
    return arr.reshape(128, KP * 2 * M).astype(E4M3)


def _prep_shared(norm_w, in_w, in_b, gate_w, gate_b, b_w, b_b, c_w, c_b, d_w, d_b,
                 out_w, out_b, a_log):
    c = np.ascontiguousarray
    f = np.float32
    a = np.exp(-np.logaddexp(0.0, a_log.astype(np.float64))).astype(f)
    in_s = _prep_dr(in_w * norm_w[None, :], S_WI)  # [128, 16*1024]
    gate_s = _prep_dr(gate_w * norm_w[None, :], S_WG)
    ig = np.stack(
        [in_s.reshape(128, NKI, KPD * 2 * 128),
         gate_s.reshape(128, NKI, KPD * 2 * 128)], axis=2
    )  # [128, mi, half, ...]
    shared = {
        "w_ig": c(ig.reshape(128, NKI * 2 * KPD * 2 * 128)),
        "w_b": _prep_dr(b_w, S_WB),
        "w_c": _prep_dr(c_w, S_WC),
        "w_d": _prep_dr(d_w, S_WD),
        "w_om": _prep_mov(out_w, S_WO),
        "bias_ig": c(np.concatenate([in_b * S_U, gate_b]).astype(f)
                     .reshape(2 * NKI, 128).T),
        "bias_bcd": c(np.concatenate(
            [b_b * BETA, c_b * (S_WC * S_U), d_b * S_Y]
        ).astype(f).reshape(3 * NKI, 128).T),
        "bias_ob": c(np.broadcast_to(out_b.astype(f), (128, DIM)).copy()),
        "a_in": c(a.reshape(NKI, 128).T),
    }
    return shared


def kernel(x, norm_w, in_w, in_b, gate_w, gate_b, b_w, b_b, c_w, c_b, d_w, d_b,
           out_w, out_b, a_log, _trace=False):
    # inputs may be jax arrays; convert up front so host math stays in numpy
    x, norm_w, in_w, in_b, gate_w, gate_b = (
        np.asarray(v, np.float32) for v in (x, norm_w, in_w, in_b, gate_w, gate_b))
    b_w, b_b, c_w, c_b, d_w, d_b, out_w, out_b, a_log = (
        np.asarray(v, np.float32)
        for v in (b_w, b_b, c_w, c_b, d_w, d_b, out_w, out_b, a_log))

    if "nc" not in _CACHED:
        # plain build: measured on HW, LDWEIGHTS pipelines under matmuls,
        # so the manifest-reordered (weight-major) schedule buys nothing
        # and its serialization costs ~40us; keep build_nc_manifest around
        # for experiments
        _CACHED["nc"] = build_nc()
    nc = _CACHED["nc"]

    shared = _prep_shared(norm_w, in_w, in_b, gate_w, gate_b, b_w, b_b, c_w, c_b,
                          d_w, d_b, out_w, out_b, a_log)
    m0_ones = np.ones((128, 256), np.float32)
    m0_reset = m0_ones.copy()
    m0_reset[:, HALO] = 0.0  # kills the recurrence carry at the true seq start
    in_maps = []
    for core in range(8):
        bi, sh = core // 2, core % 2
        m = dict(shared)
        if sh == 0:
            sl = np.concatenate(
                [np.zeros((HALO, DIM), np.float32), x[bi, 0 : S // 2]], axis=0)
            m["m0"] = m0_reset
        else:
            sl = x[bi, S // 2 - HALO : S]
            m["m0"] = m0_ones
        m["x"] = np.ascontiguousarray(sl.astype(ml_dtypes.bfloat16))
        in_maps.append(m)

    kw = {}
    if _trace:
        _install_ntff_hook()
        kw = dict(trace=True, trace_cores=[0], trace_events=False)
    res = run_bass_kernel_spmd(nc, in_maps, core_ids=list(range(8)), **kw)
    _CACHED["last_result"] = res

    outp = np.empty((B, S, DIM), np.float32)
    for core in range(8):
        bi, sh = core // 2, core % 2
        o = res.results[core]["out"]
        outp[bi, sh * (S // 2) : (sh + 1) * (S // 2)] = o[HALO : HALO + S // 2]
    return outp


# revision 64
# speedup vs baseline: 1.0014x; 1.0014x over previous
"""MinimalMambaBlock Trainium2 kernel — fp8 (e4m3) DoubleRow matmul version.

Sharding: 8 cores = 4 batch rows x 2 sequence halves. Each core processes
T = 32 + 1024 tokens of one batch row with the 32-token halo at the FRONT:
second-half cores warm the linear recurrence up through real tokens; first
half cores get 32 zero rows plus a per-core scan-mask column (m0) that
zeroes the recurrence carry exactly at the true sequence start. Every core
outputs the uniform window [32:1056).

All five projections run as fp8e4 (e4m3) matmuls in DoubleRow perf mode
(256-deep contraction per pass, 2x the fp32r MAC rate). PSUM accumulates in
fp32. Per-tensor power-of-2 scales keep operands inside e4m3 range (max 240):
  xn*16, u*32, y*64, in/gate weights *2048, b/c/d/out weights *4096.
Scale corrections fold into the existing bias/activation steps. The h scan
carries an extra beta = s_y/(s_wc*s_u) factor so phase D's
(ps_c + c_b') * h' fuses into one scalar_tensor_tensor with no extra scaling.

Device pipeline (activations in [channel, time] layout after the norm):
  A: load x [t,d] -> RMSNorm (*s_x, bf16) -> PE-transpose -> fp8 xp [d,2,t]
  B: u = (in_w @ xn + in_b) * sigmoid(gate_w @ xn + gate_b) -> fp8 u pairs
     (drains split DVE/ACT, multiplies deferred one iteration and split
      DVE/Pool so psum-slot releases never queue behind cross-engine waits)
  C: b = b_w @ u + b_b -> h' = tensor_tensor_scan(a*m0, b*beta)
  D: h' *= (c_w @ u + c_b')            (stt, in place)
  E: y = h' + (d_w @ u + d_b)*s_y      -> fp8 y pairs
  F: out[t,d] = y^T @ out_w (y stationary, token-major: no transposes)
     + (x + out_b) via one in-place stt per block; one store per token tile

Perf notes (measured on hw): 256-col psum blocks are the sweet spot (512
compiles+passes but loses PE pipeline overlap); LDWEIGHTS mostly pipelines
under matmuls so weight-major reordering is not worth it; Pool engine ops
are ~2.5x slower than DVE and cannot read PSUM.
"""

import os
import sys
from contextlib import ExitStack

import numpy as np
import ml_dtypes

sys.path.insert(0, "/opt/trn_rl_repo")

import concourse.bass as bass
import concourse.mybir as mybir
import concourse.tile as tile
from concourse.bass_utils import run_bass_kernel_spmd
from concourse.masks import make_identity

F32 = mybir.dt.float32
BF16 = mybir.dt.bfloat16
FP8 = mybir.dt.float8e4
E4M3 = ml_dtypes.float8_e4m3
AF = mybir.ActivationFunctionType
OP = mybir.AluOpType
DR = mybir.MatmulPerfMode.DoubleRow

DIM = 1024
INNER = 2048
B = 4
S = 2048
EPS = 1e-6
HALO = 32
T = 1024 + HALO  # 1056
NKD = DIM // 128  # 8 d-tiles
NKI = INNER // 128  # 16 inner tiles
KPD = NKD // 2  # 4 k-pairs over model dim
KPI = NKI // 2  # 8 k-pairs over inner dim
# Halo layout: every core's 32 halo tokens sit at the FRONT (cols 0:32).
# Second-half cores warm the recurrence up through them; first-half cores
# get 32 zero-padded rows plus a scan-reset mask column at t=32, so the
# recurrence restarts exactly at the true sequence start. The output
# window is uniformly tokens [32:1056).
# token tiles for transpose/norm (partition dim = tokens)
TTILES = [(i * 128, 128) for i in range(8)] + [(1024, HALO)]
# free-dim blocks for B/C matmuls (256-col blocks: wider 512-col blocks
# compile and pass but lose PE pipeline overlap on hw — measured slower)
TBLOCKS = [(0, 256), (256, 256), (512, 256), (768, 256), (1024, T - 1024)]
# output-window blocks for D/E/F (4 clean 256-blocks)
OBLOCKS = [(HALO, 256), (HALO + 256, 256), (HALO + 512, 256), (HALO + 768, 256)]
# output-window token tiles for phase F
FTILES = [(HALO + i * 128, 128) for i in range(8)]

# power-of-2 operand scales (validated against e4m3 max 240 on the fixed
# seed-0 inputs: scaled maxima are 87/72/60; weight bounds are exact
# 1/sqrt(fan_in) so weight maxima are static)
S_X = 16.0
S_U = 32.0
S_Y = 64.0
S_WI = 2048.0
S_WG = 2048.0
S_WB = 4096.0
S_WC = 4096.0
S_WD = 4096.0
S_WO = 4096.0
BETA = S_Y / (S_WC * S_U)  # extra scale carried by h'

_CACHED = {}


def build_nc():
    nc = bass.Bass("TRN2")

    # x is passed from the host as bf16: halves the descriptor-rate-bound
    # load time of phases A and F; the residual add keeps f32 accumulation
    x = nc.dram_tensor("x", [T, DIM], BF16, kind="ExternalInput")
    # DoubleRow weight strips, pre-laid-out host side (see _prep_shared):
    # w_ig[p, mi, half, j, i, m]; others w[p, mt, j, i, m]
    w_ig = nc.dram_tensor("w_ig", [128, NKI * 2 * KPD * 2 * 128], FP8,
                          kind="ExternalInput")
    w_b = nc.dram_tensor("w_b", [128, NKI * KPI * 2 * 128], FP8,
                         kind="ExternalInput")
    w_c = nc.dram_tensor("w_c", [128, NKI * KPI * 2 * 128], FP8,
                         kind="ExternalInput")
    w_d = nc.dram_tensor("w_d", [128, NKI * KPI * 2 * 128], FP8,
                         kind="ExternalInput")
    w_om = nc.dram_tensor("w_om", [128, KPI * 2 * DIM], FP8,
                          kind="ExternalInput")
    # per-channel vectors pre-laid-out host-side as [128, n_tiles]
    bias_ig = nc.dram_tensor("bias_ig", [128, 2 * NKI], F32, kind="ExternalInput")
    bias_bcd = nc.dram_tensor("bias_bcd", [128, 3 * NKI], F32, kind="ExternalInput")
    bias_ob = nc.dram_tensor("bias_ob", [128, DIM], F32, kind="ExternalInput")
    a_in = nc.dram_tensor("a_in", [128, NKI], F32, kind="ExternalInput")
    # per-core scan mask for block 0: ones, except first-half cores carry a
    # zero at column HALO which resets the recurrence at the true seq start
    m0_in = nc.dram_tensor("m0", [128, 256], F32, kind="ExternalInput")
    out = nc.dram_tensor("out", [T, DIM], F32, kind="ExternalOutput")

    w_ig_r = w_ig.ap().rearrange("p (mi h j i m) -> p mi h j i m",
                                 mi=NKI, h=2, j=KPD, i=2)
    w_b_r = w_b.ap().rearrange("p (mt j i m) -> p mt j i m", mt=NKI, j=KPI, i=2)
    w_c_r = w_c.ap().rearrange("p (mt j i m) -> p mt j i m", mt=NKI, j=KPI, i=2)
    w_d_r = w_d.ap().rearrange("p (mt j i m) -> p mt j i m", mt=NKI, j=KPI, i=2)
    w_om_r = w_om.ap().rearrange("p (j i d) -> p j i d", j=KPI, i=2)
    x_ap = x.ap()
    out_ap = out.ap()

    with tile.TileContext(nc) as tc, ExitStack() as ctx:
        statics = ctx.enter_context(tc.tile_pool(name="statics", bufs=1))
        xwork = ctx.enter_context(tc.tile_pool(name="xwork", bufs=2))
        wpool = ctx.enter_context(tc.tile_pool(name="wpool", bufs=4))
        work = ctx.enter_context(tc.tile_pool(name="work", bufs=3))
        small = ctx.enter_context(tc.tile_pool(name="small", bufs=8))
        frow = ctx.enter_context(tc.tile_pool(name="frow", bufs=4))
        psA = ctx.enter_context(tc.tile_pool(name="psA", bufs=1, space="PSUM"))

        identF = statics.tile([128, 128], BF16, tag="identF")
        make_identity(nc, identF)
        eps_t = statics.tile([128, 1], F32, tag="eps_t")
        nc.vector.memset(eps_t, EPS / (S_X * S_X))


        # persistent activations
        xp = [statics.tile([128, 2, T], FP8, tag=f"xp{j}", name=f"xp{j}")
              for j in range(KPD)]
        up = [statics.tile([128, 2, T], FP8, tag=f"up{j}", name=f"up{j}")
              for j in range(KPI)]
        yp = [statics.tile([128, 2, T], FP8, tag=f"yp{j}", name=f"yp{j}")
              for j in range(KPI)]
        h = [statics.tile([128, T], F32, tag=f"h{i}", name=f"h{i}")
             for i in range(NKI)]

        # ---- Phase A: load + RMSNorm (*S_X) + fp8 + transpose -> xp ----
        # x loads stay on the sync queue: spreading them across the
        # scalar/gpsimd queues was measured much slower (DMA issues block
        # those engines' compute work behind them)
        for tt, (t0, tl) in enumerate(TTILES):
            x_t = xwork.tile([128, DIM], BF16, tag="x_t")
            nc.sync.dma_start(out=x_t[:tl, :], in_=x_ap[t0 : t0 + tl, :])
            sq_t = xwork.tile([128, DIM], F32, tag="sq_t")
            sumsq = small.tile([128, 1], F32, tag="sumsq")
            nc.scalar.activation(
                sq_t[:tl, :], x_t[:tl, :], AF.Square, accum_out=sumsq[:tl, :]
            )
            rms = small.tile([128, 1], F32, tag="rms")
            # rms = sqrt(mean + eps) / S_X
            nc.scalar.activation(
                rms[:tl, :], sumsq[:tl, :], AF.Sqrt, bias=eps_t[:tl, :],
                scale=1.0 / (DIM * S_X * S_X),
            )
            scale = small.tile([128, 1], F32, tag="scale")
            nc.vector.reciprocal(scale[:tl, :], rms[:tl, :])
            xn_t = xwork.tile([128, DIM], BF16, tag="xn_t")
            nc.vector.tensor_scalar_mul(xn_t[:tl, :], x_t[:tl, :], scale[:tl, :])
            for di in range(NKD):
                tr = psA.tile([128, 128], BF16, tag="tr", bufs=2, name="tr_a")
                nc.tensor.transpose(
                    tr[:, :tl], xn_t[:tl, di * 128 : (di + 1) * 128],
                    identF[:tl, :tl],
                )
                # split the fp8 casts across DVE and ACT so neither engine
                # backs up while phase B's drains start to overlap phase A
                if di % 2 == 0:
                    nc.vector.tensor_copy(
                        xp[di // 2][:, di % 2, t0 : t0 + tl], tr[:, :tl]
                    )
                else:
                    nc.scalar.copy(
                        xp[di // 2][:, di % 2, t0 : t0 + tl], tr[:, :tl]
                    )

        # static per-channel vectors (emitted after phase A so the x-tile DMAs
        # lead the queue and the first transposes start sooner)
        b_ig = statics.tile([128, 2 * NKI], F32, tag="b_ig")
        nc.sync.dma_start(out=b_ig, in_=bias_ig.ap())
        b_bcd = statics.tile([128, 3 * NKI], F32, tag="b_bcd")
        nc.sync.dma_start(out=b_bcd, in_=bias_bcd.ap())
        a_t = statics.tile([128, NKI], F32, tag="a_t")
        nc.sync.dma_start(out=a_t, in_=a_in.ap())
        m0_t = statics.tile([128, 256], F32, tag="m0_t")
        nc.sync.dma_start(out=m0_t, in_=m0_in.ap())

        # pre-build every scan-multiplier row now: DVE has slack during
        # phase B, and phase C's critical queue then carries only the scans
        a_bcs, a_bc0s = [], []
        for ji in range(NKI):
            a_bc = small.tile([128, 256], F32, tag="a_bc", bufs=NKI)
            nc.vector.memset(a_bc, 1.0)
            nc.vector.tensor_scalar_mul(a_bc, a_bc, a_t[:, ji : ji + 1])
            a_bc0 = small.tile([128, 256], F32, tag="a_bc0", bufs=NKI // 2)
            nc.vector.tensor_scalar_mul(a_bc0, m0_t, a_t[:, ji : ji + 1])
            a_bcs.append(a_bc)
            a_bc0s.append(a_bc0)

        # ---- Phase B: u = (in @ xn + in_b) * sigmoid(gate @ xn + gate_b) ----
        # The u = u32*g multiplies are deferred by one iteration: when they
        # are emitted in program order their operands are already complete,
        # so the DVE/Pool queues never stall on a sigmoid while the next
        # iteration's psum drains sit behind them.
        def flush_mult(pmi, pu32, pgs):
            for bi, (n0, nl) in enumerate(TBLOCKS):
                eng = nc.vector if bi < 3 else nc.gpsimd
                eng.tensor_mul(
                    up[pmi // 2][:, pmi % 2, n0 : n0 + nl],
                    pu32[:, n0 : n0 + nl], pgs[bi][:, :nl],
                )

        pending = None
        for mi in range(NKI):
            w_s = wpool.tile([128, 2, KPD, 2, 128], FP8, tag="ws", name="w_ig_s")
            nc.sync.dma_start(out=w_s, in_=w_ig_r[:, mi])
            ps_us = [psA.tile([128, nl], F32, tag=("ps" if bi < 4 else "tr"),
                              bufs=(6 if bi < 4 else 2), name=f"ps_u{bi}")
                     for bi, (n0, nl) in enumerate(TBLOCKS)]
            for j in range(KPD):
                for bi, (n0, nl) in enumerate(TBLOCKS):
                    nc.tensor.matmul(
                        ps_us[bi], w_s[:, 0, j], xp[j][:, :, n0 : n0 + nl],
                        start=(j == 0), stop=(j == KPD - 1), perf_mode=DR,
                    )
            u32 = work.tile([128, T], F32, tag="fullT", name="u32")
            for bi, (n0, nl) in enumerate(TBLOCKS):
                # drains gate the gate-matmuls' psum slots; split DVE/ACT so
                # neither engine serializes the pipeline (gpsimd can't read
                # PSUM)
                if bi < 3:
                    nc.vector.tensor_scalar(
                        u32[:, n0 : n0 + nl], ps_us[bi],
                        S_U / (S_WI * S_X), b_ig[:, mi : mi + 1],
                        op0=OP.mult, op1=OP.add,
                    )
                else:
                    nc.scalar.activation(
                        u32[:, n0 : n0 + nl], ps_us[bi], AF.Identity,
                        bias=b_ig[:, mi : mi + 1], scale=S_U / (S_WI * S_X),
                    )
            ps_gs = [psA.tile([128, nl], F32, tag=("ps" if bi < 4 else "tr"),
                              bufs=(6 if bi < 4 else 2), name=f"ps_g{bi}")
                     for bi, (n0, nl) in enumerate(TBLOCKS)]
            for j in range(KPD):
                for bi, (n0, nl) in enumerate(TBLOCKS):
                    nc.tensor.matmul(
                        ps_gs[bi], w_s[:, 1, j], xp[j][:, :, n0 : n0 + nl],
                        start=(j == 0), stop=(j == KPD - 1), perf_mode=DR,
                    )
            g_sbs = []
            for bi, (n0, nl) in enumerate(TBLOCKS):
                g_sb = small.tile([128, 256], F32, tag="g_sb", bufs=10)
                nc.scalar.activation(
                    g_sb[:, :nl], ps_gs[bi], AF.Sigmoid,
                    bias=b_ig[:, NKI + mi : NKI + mi + 1],
                    scale=1.0 / (S_WG * S_X),
                )
                g_sbs.append(g_sb)
            if pending is not None:
                flush_mult(*pending)
            pending = (mi, u32, g_sbs)
        flush_mult(*pending)

        # ---- Phase C: b = b_w @ u + b_b ; h' = scan(a, b*BETA) ----
        for ji in range(NKI):
            w_s = wpool.tile([128, KPI, 2, 128], FP8, tag="ws", name="w_b_s")
            nc.sync.dma_start(out=w_s, in_=w_b_r[:, ji])
            pss = [psA.tile([128, nl], F32, tag=("ps" if bi < 4 else "tr"),
                            bufs=(6 if bi < 4 else 2), name=f"ps_b{bi}")
                   for bi, (n0, nl) in enumerate(TBLOCKS)]
            for j in range(KPI):
                for bi, (n0, nl) in enumerate(TBLOCKS):
                    nc.tensor.matmul(
                        pss[bi], w_s[:, j], up[j][:, :, n0 : n0 + nl],
                        start=(j == 0), stop=(j == KPI - 1), perf_mode=DR,
                    )
            b_full = work.tile([128, T], F32, tag="fullT", name="b_full")
            for bi, (n0, nl) in enumerate(TBLOCKS):
                nc.scalar.activation(
                    b_full[:, n0 : n0 + nl], pss[bi], AF.Identity,
                    bias=b_bcd[:, ji : ji + 1], scale=BETA / (S_WB * S_U),
                )
            a_bc = a_bcs[ji]
            a_bc0 = a_bc0s[ji]  # block 0 carries the per-core reset mask
            for bi, (n0, nl) in enumerate(TBLOCKS):
                init = 0.0 if bi == 0 else h[ji][:, n0 - 1 : n0]
                nc.vector.tensor_tensor_scan(
                    h[ji][:, n0 : n0 + nl],
                    (a_bc0 if bi == 0 else a_bc)[:, :nl],
                    b_full[:, n0 : n0 + nl], init, op0=OP.mult, op1=OP.add,
                )

        # ---- Phase D: h' *= (c_w @ u + c_b')   (in place) ----
        for ji in range(NKI):
            w_s = wpool.tile([128, KPI, 2, 128], FP8, tag="ws", name="w_c_s")
            nc.sync.dma_start(out=w_s, in_=w_c_r[:, ji])
            pss = [psA.tile([128, nl], F32, tag="ps", bufs=6, name=f"ps_c{bi}")
                   for bi, (n0, nl) in enumerate(OBLOCKS)]
            for j in range(KPI):
                for bi, (n0, nl) in enumerate(OBLOCKS):
                    nc.tensor.matmul(
                        pss[bi], w_s[:, j], up[j][:, :, n0 : n0 + nl],
                        start=(j == 0), stop=(j == KPI - 1), perf_mode=DR,
                    )
            for bi, (n0, nl) in enumerate(OBLOCKS):
                nc.vector.scalar_tensor_tensor(
                    h[ji][:, n0 : n0 + nl], pss[bi],
                    b_bcd[:, NKI + ji : NKI + ji + 1], h[ji][:, n0 : n0 + nl],
                    op0=OP.add, op1=OP.mult,
                )

        # phase-F statics, emitted late so the 2MB transfer rides under
        # phases D/E instead of competing with phase A's x loads
        w_om_t = statics.tile([128, KPI, 2, DIM], FP8, tag="w_om")
        nc.gpsimd.dma_start(out=w_om_t, in_=w_om_r)
        bias_ob_t = statics.tile([128, DIM], F32, tag="bias_ob")
        nc.gpsimd.dma_start(out=bias_ob_t, in_=bias_ob.ap())

        # ---- Phase E: y = h' + (d_w @ u + d_b)*S_Y  -> fp8 yp ----
        for ji in range(NKI):
            w_s = wpool.tile([128, KPI, 2, 128], FP8, tag="ws", name="w_d_s")
            nc.sync.dma_start(out=w_s, in_=w_d_r[:, ji])
            pss = [psA.tile([128, nl], F32, tag="ps", bufs=6, name=f"ps_d{bi}")
                   for bi, (n0, nl) in enumerate(OBLOCKS)]
            for j in range(KPI):
                for bi, (n0, nl) in enumerate(OBLOCKS):
                    nc.tensor.matmul(
                        pss[bi], w_s[:, j], up[j][:, :, n0 : n0 + nl],
                        start=(j == 0), stop=(j == KPI - 1), perf_mode=DR,
                    )
            dd32 = work.tile([128, T], F32, tag="fullT", name="dd32")
            for bi, (n0, nl) in enumerate(OBLOCKS):
                nc.scalar.activation(
                    dd32[:, n0 : n0 + nl], pss[bi], AF.Identity,
                    bias=b_bcd[:, 2 * NKI + ji : 2 * NKI + ji + 1],
                    scale=S_Y / (S_WD * S_U),
                )
            for bi, (n0, nl) in enumerate(OBLOCKS):
                nc.vector.tensor_add(
                    yp[ji // 2][:, ji % 2, n0 : n0 + nl],
                    h[ji][:, n0 : n0 + nl], dd32[:, n0 : n0 + nl],
                )

        # ---- Phase F: out[t, d] = y^T @ out_w + out_b + x  (token-major;
        # y is the matmul stationary so no transposes are needed, the bias
        # folds into the residual row, and each token tile streams its own
        # store as soon as it completes) ----
        for tt, (t0, tl) in enumerate(FTILES):
            x_r = statics.tile([128, DIM], BF16, tag=f"h{tt}",
                               name=f"x_row{tt}")
            (nc.gpsimd if tt % 2 else nc.sync).dma_start(
                out=x_r[:tl, :], in_=x_ap[t0 : t0 + tl, :])
            xb = statics.tile([128, DIM], F32, tag=f"h{8 + tt}", name=f"xb{tt}")
            nc.vector.tensor_add(xb, x_r, bias_ob_t)
            pss = [psA.tile([128, 256], F32, tag="ps", bufs=6, name=f"ps_o{bi}")
                   for bi in range(4)]
            for j in range(KPI):
                for bi in range(4):
                    nc.tensor.matmul(
                        pss[bi], yp[j][:, :, t0 : t0 + tl],
                        w_om_t[:, j, :, bi * 256 : (bi + 1) * 256],
                        start=(j == 0), stop=(j == KPI - 1), perf_mode=DR,
                    )
            for bi in range(4):
                nc.vector.scalar_tensor_tensor(
                    xb[:, bi * 256 : (bi + 1) * 256], pss[bi],
                    1.0 / (S_WO * S_Y), xb[:, bi * 256 : (bi + 1) * 256],
                    op0=OP.mult, op1=OP.add,
                )
            (nc.gpsimd if tt % 2 else nc.sync).dma_start(
                out=out_ap[t0 : t0 + tl, :], in_=xb[:tl, :])

    # walrus in this container only encodes 1 sync-wait on CTRL instructions
    from birfix_embed import patch_nc

    patch_nc(nc)
    return nc


# ---- embedded birfix (kernel.py must be self-contained) ----
def _enable_ldw_opt():
    """Flip walrus --enable-ldw-opt so consecutive same-weight matmuls skip
    the redundant LDWEIGHTS reload."""
    from concourse import bass_utils as _bu

    if getattr(_bu, "_ldw_opt_patched", False):
        return
    _orig = _bu.run_command

    def patched(argv, **kw):
        argv = ["--enable-ldw-opt=true" if a == "--enable-ldw-opt=false" else a
                for a in argv]
        return _orig(argv, **kw)

    _bu.run_command = patched
    _bu._ldw_opt_patched = True


# NOTE: not enabled — the Tile legalizer splits fp8 DoubleRow matmuls into
# explicit Ldweights+Matmult, and walrus rejects standalone Ldweights when
# --enable-ldw-opt=true. Ldweights dedup happens at legalize time instead.
# _enable_ldw_opt()


def _install_birfix():
    import json as _json
    import types

    mod = types.ModuleType("birfix_embed")

    CTRL = {"Drain", "NoOp", "EventSemaphore", "TriggeredCopy", "RegisterMove",
            "UnconditionalBranch", "Halt"}
    MAX_COMPUTE_WAITS = 1

    def dedup_ldweights(d):
        """Drop Ldweights whose stationary operand is already loaded.

        The Tile legalizer emits one Ldweights per (DoubleRow) Matmult; the
        PE array keeps its stationary across matmuls, so within a run of
        same-weight matmuls only the first load is needed. Any transpose or
        self-loading Matmult clobbers the array and resets tracking. The BIR
        here is post-schedule, so per-engine order is final."""
        removed = 0
        for fn in d.get("functions", []):
            for bb in fn.get("blocks", fn.get("basicblocks", [])):
                insts = bb.get("instructions", [])
                out = []
                loaded = None
                for inst in insts:
                    if inst.get("engine") != "PE":
                        out.append(inst)
                        continue
                    op = inst.get("opcode")
                    if op == "Ldweights":
                        sig = _json.dumps(
                            [inst.get("ins"), inst.get("perf_mode"),
                             inst.get("tile_position"), inst.get("tile_size"),
                             inst.get("is_transpose")],
                            sort_keys=True,
                        )
                        sync = inst.get("sync_info") or {}
                        if sig == loaded and not sync.get("on_update"):
                            waits = sync.get("on_wait") or []
                            if waits:
                                out.append({
                                    "engine": "PE", "ins": [],
                                    "name": inst["name"] + "_dd",
                                    "opcode": "NoOp", "outs": [],
                                    "sync_info": {"on_update": [],
                                                  "on_wait": waits},
                                })
                            removed += 1
                            continue
                        loaded = sig
                        out.append(inst)
                    elif op == "Matmult":
                        if inst.get("is_transpose") or inst.get("ldweights", True):
                            loaded = None
                        out.append(inst)
                    else:
                        out.append(inst)
                bb["instructions"] = out
        return removed

    def fix_bir_json(bir, max_ctrl=1, max_compute=MAX_COMPUTE_WAITS):
        d = _json.loads(bir)
        n_removed = dedup_ldweights(d)
        sys.stderr.write(f"birfix: removed {n_removed} redundant Ldweights\n")
        n_split = 0
        for fn in d.get("functions", []):
            for bb in fn.get("blocks", fn.get("basicblocks", [])):
                insts = bb.get("instructions", [])
                out = []
                changed = False
                for inst in insts:
                    sync = inst.get("sync_info")
                    cap = max_ctrl if inst.get("opcode") in CTRL else max_compute
                    if sync and len(sync.get("on_wait") or []) > cap:
                        waits = sync["on_wait"]
                        keep = waits[-cap:]
                        extra = waits[:-cap]
                        for i in range(0, len(extra), max_ctrl):
                            out.append(
                                {
                                    "engine": inst["engine"],
                                    "ins": [],
                                    "name": inst["name"] + f"_ws{i}",
                                    "opcode": "NoOp",
                                    "outs": [],
                                    "sync_info": {
                                        "on_update": [],
                                        "on_wait": extra[i : i + max_ctrl],
                                    },
                                }
                            )
                            n_split += 1
                        sync["on_wait"] = keep
                        changed = True
                    out.append(inst)
                if changed:
                    bb["instructions"] = out
        return _json.dumps(d).encode(), n_split

    def patch_nc(nc, max_ctrl=1, max_compute=MAX_COMPUTE_WAITS):
        orig = nc.to_json_bytes

        def patched():
            fixed, _ = fix_bir_json(orig(), max_ctrl, max_compute)
            return fixed

        nc.to_json_bytes = patched
        return nc

    mod.fix_bir_json = fix_bir_json
    mod.patch_nc = patch_nc
    sys.modules["birfix_embed"] = mod


_install_birfix()


def _install_ntff_hook():
    """The image lacks antenv.axon_hooks; recreate it so trace=True works."""
    import types

    if "antenv.axon_hooks" in sys.modules:
        return
    try:
        from trn_agent_boot.trn_boot import _ntff_profile_via_ctypes

        hook = _ntff_profile_via_ctypes("/opt/axon/libaxon_pjrt.so")
    except Exception:
        hook = None
    mod = types.ModuleType("antenv.axon_hooks")
    mod.get_axon_ntff_profile_hook = lambda: hook
    mod.set_axon_ntff_profile_hook = lambda h: None
    sys.modules["antenv.axon_hooks"] = mod


# ---- two-pass build: capture schedule manifest, reorder matmuls to
# weight-major (dependency- and slot-safe), rebuild with the manifest ----
def _fishpath_compat():
    from concourse._compat import FishPath

    if not hasattr(FishPath, "open"):
        def _open(self, mode="r"):
            if "w" in mode:
                self._path.parent.mkdir(parents=True, exist_ok=True)
            return open(self._path, mode)
        FishPath.open = _open
    if not hasattr(FishPath, "makedirs"):
        FishPath.makedirs = (
            lambda self: self._path.mkdir(parents=True, exist_ok=True))
    if not hasattr(FishPath, "is_file"):
        FishPath.is_file = lambda self: self._path.is_file()
    if not hasattr(FishPath, "parent"):
        FishPath.parent = property(lambda self: FishPath(self._path.parent))
    if not hasattr(FishPath, "__fspath__"):
        FishPath.__fspath__ = lambda self: str(self._path)


def _rewrite_manifest(mdir, bir, releases):
    """Reorder the captured manifest so DoubleRow matmuls run weight-major
    (j-outer), respecting data deps and tile slot reuse, so the birfix
    Ldweights dedup can drop redundant PE weight loads."""
    import glob as _glob
    import heapq
    import json as _json
    from collections import defaultdict

    mpath = _glob.glob(os.path.join(mdir, "*.json"))[0]
    with open(mpath) as f:
        m = _json.load(f)
    dpath = _glob.glob(os.path.join(mdir, "*_debug_info/instruction_deps.json"))[0]
    with open(dpath) as f:
        deps = _json.load(f)

    meta = {}
    readers = defaultdict(set)
    writers = defaultdict(set)
    for fn in bir.get("functions", []):
        for bb in fn.get("blocks", fn.get("basicblocks", [])):
            for inst in bb.get("instructions", []):
                nm = inst.get("name")
                for a in inst.get("ins", []) or []:
                    if isinstance(a, dict) and a.get("memref"):
                        readers[a["memref"]].add(nm)
                for a in inst.get("outs", []) or []:
                    if isinstance(a, dict) and a.get("memref"):
                        writers[a["memref"]].add(nm)
                if (inst.get("opcode") == "Matmult"
                        and not inst.get("is_transpose")
                        and inst.get("perf_mode") == "DoubleRow"):
                    wap = inst["ins"][1]
                    meta[nm] = (wap["memref"], wap["offset"])
    for relname, tname in releases.items():
        readers[tname].add(relname)

    slot_groups = defaultdict(list)
    for tname, (addr, space) in m["addresses"].items():
        slot_groups[(space, addr)].append(tname)

    def alloc_id(tname):
        try:
            return int(tname.rsplit("_", 1)[1])
        except ValueError:
            return 0

    for block, order in m["order"].items():
        rank = {}
        groups = defaultdict(list)
        for pos, e in enumerate(order):
            rank[e["name"]] = pos
            if e["engine"] == "PE" and e["name"] in meta:
                groups[meta[e["name"]][0]].append(pos)
        for w, positions in groups.items():
            tagged = sorted(
                (meta[order[p]["name"]][1], i, order[p]["name"])
                for i, p in enumerate(positions)
            )
            for p, (_, _, name) in zip(positions, tagged):
                rank[name] = p
        # emit all non-PE entries (drains, DMAs, virtual releases) as soon as
        # they are ready — their original positions are stale after the
        # matmul permutation and would otherwise delay psum-slot releases,
        # breaking the same-weight matmul runs the dedup relies on
        for e in order:
            if e["engine"] != "PE":
                rank[e["name"]] = -1
        for relname in releases:
            if relname in rank:
                rank[relname] = -1

        entry_by_name = {e["name"]: e for e in order}
        succ = defaultdict(list)
        indeg = {e["name"]: 0 for e in order}
        edges = set()

        def add_edge(a, b):
            if a != b and (a, b) not in edges:
                edges.add((a, b))
                succ[a].append(b)
                indeg[b] += 1

        for name, dd in deps.items():
            if name not in indeg:
                continue
            for pred in set(dd.get("pre_data", []) + dd.get("pre_no_sync", [])):
                if pred in indeg:
                    add_edge(pred, name)
        for (space, addr), tiles in slot_groups.items():
            if len(tiles) < 2:
                continue
            tiles = sorted(tiles, key=alloc_id)
            for t1, t2 in zip(tiles, tiles[1:]):
                uses = (readers[t1] | writers[t1]) & indeg.keys()
                wrts = writers[t2] & indeg.keys()
                for u in uses:
                    for wv in wrts:
                        add_edge(u, wv)
        # chain weight-groups so a group's matmuls finish before the next
        # group starts — keeps same-weights matmuls consecutive on the PE
        # queue (the Ldweights dedup then drops ~4/5 of the weight loads)
        glist = sorted(groups.items(), key=lambda kv: min(kv[1]))
        for (w1, p1), (w2, p2) in zip(glist, glist[1:]):
            last = max(p1, key=lambda p: rank[order[p]["name"]])
            first = min(p2, key=lambda p: rank[order[p]["name"]])
            add_edge(order[last]["name"], order[first]["name"])

        heap = [(rank[nm], nm) for nm, c in indeg.items() if c == 0]
        heapq.heapify(heap)
        new_order = []
        while heap:
            _, nm = heapq.heappop(heap)
            new_order.append(entry_by_name[nm])
            for s in succ[nm]:
                indeg[s] -= 1
                if indeg[s] == 0:
                    heapq.heappush(heap, (rank[s], s))
        assert len(new_order) == len(order), (len(new_order), len(order))
        m["order"][block] = new_order

    with open(mpath, "w") as f:
        _json.dump(m, f)


def build_nc_manifest():
    import json as _json
    import shutil
    import tempfile

    _fishpath_compat()
    mdir = tempfile.mkdtemp(prefix="bass_manifest_")
    saved = {k: os.environ.get(k) for k in
             ("TILE_CAPTURE_MANIFEST_PATH", "TILE_SCHEDULER",
              "TILE_LOAD_MANIFEST_PATH")}
    try:
        os.environ["TILE_CAPTURE_MANIFEST_PATH"] = mdir
        os.environ.pop("TILE_SCHEDULER", None)
        os.environ.pop("TILE_LOAD_MANIFEST_PATH", None)
        nc1 = build_nc()
        bir = _json.loads(nc1.to_json_bytes())
        releases = {}
        for nm, inst in nc1.inst_map.items():
            if (type(inst).__name__ == "BassTileRelease"
                    and inst.bass_tile is not None):
                releases[nm] = inst.bass_tile.name
        _rewrite_manifest(mdir, bir, releases)
        del nc1, bir
        os.environ.pop("TILE_CAPTURE_MANIFEST_PATH", None)
        os.environ["TILE_SCHEDULER"] = "manifest"
        os.environ["TILE_LOAD_MANIFEST_PATH"] = mdir
        nc2 = build_nc()
        return nc2
    finally:
        for k, v in saved.items():
            if v is None:
                os.environ.pop(k, None)
            else:
                os.environ[k] = v
        shutil.rmtree(mdir, ignore_errors=True)


def _prep_dr(W, s):
    """[M, K] weight -> DoubleRow strip layout [128, MT*KP*2*128] fp8,
    where strip[p, mt, j, i, m] = (W*s)[mt*128+m, j*256+i*128+p]."""
    M, K = W.shape
    MT, KP = M // 128, K // 256
    Wq = np.clip(W.astype(np.float64) * s, -240.0, 240.0)
    arr = np.ascontiguousarray(Wq.T).reshape(KP, 2, 128, MT, 128)
    arr = np.ascontiguousarray(arr.transpose(2, 3, 0, 1, 4))  # p mt j i m
    return arr.reshape(128, MT * KP * 2 * 128).astype(E4M3)


def _prep_mov(W, s):
    """[M, K] weight -> DoubleRow moving layout [128, KP*2*M] fp8,
    where mov[p, j, i, d] = (W*s)[d, j*256+i*128+p]."""
    M, K = W.shape
    KP = K // 256
    Wq = np.clip(W.astype(np.float64) * s, -240.0, 240.0)
    arr = np.ascontiguousarray(Wq.T).reshape(KP, 2, 128, M)
    arr = np.ascontiguousarray(arr.transpose(2, 0, 1, 3))  # p j i d
    return arr.reshape(128, KP * 2 * M).astype(E4M3)


def _prep_shared(norm_w, in_w, in_b, gate_w, gate_b, b_w, b_b, c_w, c_b, d_w, d_b,
                 out_w, out_b, a_log):
    c = np.ascontiguousarray
    f = np.float32
    a = np.exp(-np.logaddexp(0.0, a_log.astype(np.float64))).astype(f)
    in_s = _prep_dr(in_w * norm_w[None, :], S_WI)  # [128, 16*1024]
    gate_s = _prep_dr(gate_w * norm_w[None, :], S_WG)
    ig = np.stack(
        [in_s.reshape(128, NKI, KPD * 2 * 128),
         gate_s.reshape(128, NKI, KPD * 2 * 128)], axis=2
    )  # [128, mi, half, ...]
    shared = {
        "w_ig": c(ig.reshape(128, NKI * 2 * KPD * 2 * 128)),
        "w_b": _prep_dr(b_w, S_WB),
        "w_c": _prep_dr(c_w, S_WC),
        "w_d": _prep_dr(d_w, S_WD),
        "w_om": _prep_mov(out_w, S_WO),
        "bias_ig": c(np.concatenate([in_b * S_U, gate_b]).astype(f)
                     .reshape(2 * NKI, 128).T),
        "bias_bcd": c(np.concatenate(
            [b_b * BETA, c_b * (S_WC * S_U), d_b * S_Y]
        ).astype(f).reshape(3 * NKI, 128).T),
        "bias_ob": c(np.broadcast_to(out_b.astype(f), (128, DIM)).copy()),
        "a_in": c(a.reshape(NKI, 128).T),
    }
    return shared


def kernel(x, norm_w, in_w, in_b, gate_w, gate_b, b_w, b_b, c_w, c_b, d_w, d_b,
           out_w, out_b, a_log, _trace=False):
    # inputs may be jax arrays; convert up front so host math stays in numpy
    x, norm_w, in_w, in_b, gate_w, gate_b = (
        np.asarray(v, np.float32) for v in (x, norm_w, in_w, in_b, gate_w, gate_b))
    b_w, b_b, c_w, c_b, d_w, d_b, out_w, out_b, a_log = (
        np.asarray(v, np.float32)
        for v in (b_w, b_b, c_w, c_b, d_w, d_b, out_w, out_b, a_log))

    if "nc" not in _CACHED:
        # plain build: measured on HW, LDWEIGHTS pipelines under matmuls,
        # so the manifest-reordered (weight-major) schedule buys nothing
        # and its serialization costs ~40us; keep build_nc_manifest around
        # for experiments
        _CACHED["nc"] = build_nc()
    nc = _CACHED["nc"]

    shared = _prep_shared(norm_w, in_w, in_b, gate_w, gate_b, b_w, b_b, c_w, c_b,
                          d_w, d_b, out_w, out_b, a_log)
    m0_ones = np.ones((128, 256), np.float32)
    m0_reset = m0_ones.copy()
    m0_reset[:, HALO] = 0.0  # kills the recurrence carry at the true seq start
    in_maps = []
    for core in range(8):
        bi, sh = core // 2, core % 2
        m = dict(shared)
        if sh == 0:
            sl = np.concatenate(
                [np.zeros((HALO, DIM), np.float32), x[bi, 0 : S // 2]], axis=0)
            m["m0"] = m0_reset
        else:
            sl = x[bi, S // 2 - HALO : S]
            m["m0"] = m0_ones
        m["x"] = np.ascontiguousarray(sl.astype(ml_dtypes.bfloat16))
        in_maps.append(m)

    kw = {}
    if _trace:
        _install_ntff_hook()
        kw = dict(trace=True, trace_cores=[0], trace_events=False)
    res = run_bass_kernel_spmd(nc, in_maps, core_ids=list(range(8)), **kw)
    _CACHED["last_result"] = res

    outp = np.empty((B, S, DIM), np.float32)
    for core in range(8):
        bi, sh = core // 2, core % 2
        o = res.results[core]["out"]
        outp[bi, sh * (S // 2) : (sh + 1) * (S // 2)] = o[HALO : HALO + S // 2]
    return outp


# revision 65
# speedup vs baseline: 1.0073x; 1.0059x over previous
"""MinimalMambaBlock Trainium2 kernel — fp8 (e4m3) DoubleRow matmul version.

Sharding: 8 cores = 4 batch rows x 2 sequence halves. Each core processes
T = 32 + 1024 tokens of one batch row with the 32-token halo at the FRONT:
second-half cores warm the linear recurrence up through real tokens; first
half cores get 32 zero rows plus a per-core scan-mask column (m0) that
zeroes the recurrence carry exactly at the true sequence start. Every core
outputs the uniform window [32:1056).

All five projections run as fp8e4 (e4m3) matmuls in DoubleRow perf mode
(256-deep contraction per pass, 2x the fp32r MAC rate). PSUM accumulates in
fp32. Per-tensor power-of-2 scales keep operands inside e4m3 range (max 240):
  xn*16, u*32, y*64, in/gate weights *2048, b/c/d/out weights *4096.
Scale corrections fold into the existing bias/activation steps. The h scan
carries an extra beta = s_y/(s_wc*s_u) factor so phase D's
(ps_c + c_b') * h' fuses into one scalar_tensor_tensor with no extra scaling.

Device pipeline (activations in [channel, time] layout after the norm):
  A: load x [t,d] -> RMSNorm (*s_x, bf16) -> PE-transpose -> fp8 xp [d,2,t]
  B: u = (in_w @ xn + in_b) * sigmoid(gate_w @ xn + gate_b) -> fp8 u pairs
     (drains split DVE/ACT, multiplies deferred one iteration and split
      DVE/Pool so psum-slot releases never queue behind cross-engine waits)
  C: b = b_w @ u + b_b -> h' = tensor_tensor_scan(a*m0, b*beta)
  D: h' *= (c_w @ u + c_b')            (stt, in place)
  E: y = h' + (d_w @ u + d_b)*s_y      -> fp8 y pairs
  F: out[t,d] = y^T @ out_w (y stationary, token-major: no transposes)
     + (x + out_b) via one in-place stt per block; one store per token tile

Perf notes (measured on hw): 256-col psum blocks are the sweet spot (512
compiles+passes but loses PE pipeline overlap); LDWEIGHTS mostly pipelines
under matmuls so weight-major reordering is not worth it; Pool engine ops
are ~2.5x slower than DVE and cannot read PSUM.
"""

import os
import sys
from contextlib import ExitStack

import numpy as np
import ml_dtypes

sys.path.insert(0, "/opt/trn_rl_repo")

import concourse.bass as bass
import concourse.mybir as mybir
import concourse.tile as tile
from concourse.bass_utils import run_bass_kernel_spmd
from concourse.masks import make_identity

F32 = mybir.dt.float32
BF16 = mybir.dt.bfloat16
FP8 = mybir.dt.float8e4
E4M3 = ml_dtypes.float8_e4m3
AF = mybir.ActivationFunctionType
OP = mybir.AluOpType
DR = mybir.MatmulPerfMode.DoubleRow

DIM = 1024
INNER = 2048
B = 4
S = 2048
EPS = 1e-6
HALO = 32
T = 1024 + HALO  # 1056
NKD = DIM // 128  # 8 d-tiles
NKI = INNER // 128  # 16 inner tiles
KPD = NKD // 2  # 4 k-pairs over model dim
KPI = NKI // 2  # 8 k-pairs over inner dim
# Halo layout: every core's 32 halo tokens sit at the FRONT (cols 0:32).
# Second-half cores warm the recurrence up through them; first-half cores
# get 32 zero-padded rows plus a scan-reset mask column at t=32, so the
# recurrence restarts exactly at the true sequence start. The output
# window is uniformly tokens [32:1056).
# token tiles for transpose/norm (partition dim = tokens)
TTILES = [(i * 128, 128) for i in range(8)] + [(1024, HALO)]
# free-dim blocks for B/C matmuls (256-col blocks: wider 512-col blocks
# compile and pass but lose PE pipeline overlap on hw — measured slower)
TBLOCKS = [(0, 256), (256, 256), (512, 256), (768, 256), (1024, T - 1024)]
# output-window blocks for D/E/F (4 clean 256-blocks)
OBLOCKS = [(HALO, 256), (HALO + 256, 256), (HALO + 512, 256), (HALO + 768, 256)]
# output-window token tiles for phase F
FTILES = [(HALO + i * 128, 128) for i in range(8)]

# power-of-2 operand scales (validated against e4m3 max 240 on the fixed
# seed-0 inputs: scaled maxima are 87/72/60; weight bounds are exact
# 1/sqrt(fan_in) so weight maxima are static)
S_X = 16.0
S_U = 32.0
S_Y = 64.0
S_WI = 2048.0
S_WG = 2048.0
S_WB = 4096.0
S_WC = 4096.0
S_WD = 4096.0
S_WO = 4096.0
BETA = S_Y / (S_WC * S_U)  # extra scale carried by h'

_CACHED = {}


def build_nc():
    nc = bass.Bass("TRN2")

    # x is passed from the host as bf16: halves the descriptor-rate-bound
    # load time of phases A and F; the residual add keeps f32 accumulation
    x = nc.dram_tensor("x", [T, DIM], BF16, kind="ExternalInput")
    # DoubleRow weight strips, pre-laid-out host side (see _prep_shared):
    # w_ig[p, mi, half, j, i, m]; others w[p, mt, j, i, m]
    w_ig = nc.dram_tensor("w_ig", [128, NKI * 2 * KPD * 2 * 128], FP8,
                          kind="ExternalInput")
    w_b = nc.dram_tensor("w_b", [128, NKI * KPI * 2 * 128], FP8,
                         kind="ExternalInput")
    w_c = nc.dram_tensor("w_c", [128, NKI * KPI * 2 * 128], FP8,
                         kind="ExternalInput")
    w_d = nc.dram_tensor("w_d", [128, NKI * KPI * 2 * 128], FP8,
                         kind="ExternalInput")
    w_om = nc.dram_tensor("w_om", [128, KPI * 2 * DIM], FP8,
                          kind="ExternalInput")
    # per-channel vectors pre-laid-out host-side as [128, n_tiles]
    bias_ig = nc.dram_tensor("bias_ig", [128, 2 * NKI], F32, kind="ExternalInput")
    bias_bcd = nc.dram_tensor("bias_bcd", [128, 3 * NKI], F32, kind="ExternalInput")
    bias_ob = nc.dram_tensor("bias_ob", [128, DIM], F32, kind="ExternalInput")
    a_in = nc.dram_tensor("a_in", [128, NKI], F32, kind="ExternalInput")
    # per-core scan mask for block 0: ones, except first-half cores carry a
    # zero at column HALO which resets the recurrence at the true seq start
    m0_in = nc.dram_tensor("m0", [128, 256], F32, kind="ExternalInput")
    out = nc.dram_tensor("out", [T, DIM], F32, kind="ExternalOutput")

    w_ig_r = w_ig.ap().rearrange("p (mi h j i m) -> p mi h j i m",
                                 mi=NKI, h=2, j=KPD, i=2)
    w_b_r = w_b.ap().rearrange("p (mt j i m) -> p mt j i m", mt=NKI, j=KPI, i=2)
    w_c_r = w_c.ap().rearrange("p (mt j i m) -> p mt j i m", mt=NKI, j=KPI, i=2)
    w_d_r = w_d.ap().rearrange("p (mt j i m) -> p mt j i m", mt=NKI, j=KPI, i=2)
    w_om_r = w_om.ap().rearrange("p (j i d) -> p j i d", j=KPI, i=2)
    x_ap = x.ap()
    out_ap = out.ap()

    with tile.TileContext(nc) as tc, ExitStack() as ctx:
        statics = ctx.enter_context(tc.tile_pool(name="statics", bufs=1))
        xwork = ctx.enter_context(tc.tile_pool(name="xwork", bufs=4))
        wpool = ctx.enter_context(tc.tile_pool(name="wpool", bufs=4))
        work = ctx.enter_context(tc.tile_pool(name="work", bufs=3))
        small = ctx.enter_context(tc.tile_pool(name="small", bufs=8))
        frow = ctx.enter_context(tc.tile_pool(name="frow", bufs=4))
        psA = ctx.enter_context(tc.tile_pool(name="psA", bufs=1, space="PSUM"))

        identF = statics.tile([128, 128], BF16, tag="identF")
        make_identity(nc, identF)
        eps_t = statics.tile([128, 1], F32, tag="eps_t")
        nc.vector.memset(eps_t, EPS / (S_X * S_X))


        # persistent activations
        xp = [statics.tile([128, 2, T], FP8, tag=f"xp{j}", name=f"xp{j}")
              for j in range(KPD)]
        up = [statics.tile([128, 2, T], FP8, tag=f"up{j}", name=f"up{j}")
              for j in range(KPI)]
        yp = [statics.tile([128, 2, T], FP8, tag=f"yp{j}", name=f"yp{j}")
              for j in range(KPI)]
        h = [statics.tile([128, T], F32, tag=f"h{i}", name=f"h{i}")
             for i in range(NKI)]

        # ---- Phase A: load + RMSNorm (*S_X) + fp8 + transpose -> xp ----
        # x loads stay on the sync queue: spreading them across the
        # scalar/gpsimd queues was measured much slower (DMA issues block
        # those engines' compute work behind them)
        for tt, (t0, tl) in enumerate(TTILES):
            x_t = xwork.tile([128, DIM], BF16, tag="x_t")
            nc.sync.dma_start(out=x_t[:tl, :], in_=x_ap[t0 : t0 + tl, :])
            sq_t = xwork.tile([128, DIM], F32, tag="sq_t")
            sumsq = small.tile([128, 1], F32, tag="sumsq")
            nc.scalar.activation(
                sq_t[:tl, :], x_t[:tl, :], AF.Square, accum_out=sumsq[:tl, :]
            )
            rms = small.tile([128, 1], F32, tag="rms")
            # rms = sqrt(mean + eps) / S_X
            nc.scalar.activation(
                rms[:tl, :], sumsq[:tl, :], AF.Sqrt, bias=eps_t[:tl, :],
                scale=1.0 / (DIM * S_X * S_X),
            )
            scale = small.tile([128, 1], F32, tag="scale")
            nc.vector.reciprocal(scale[:tl, :], rms[:tl, :])
            xn_t = xwork.tile([128, DIM], BF16, tag="xn_t")
            nc.vector.tensor_scalar_mul(xn_t[:tl, :], x_t[:tl, :], scale[:tl, :])
            for di in range(NKD):
                tr = psA.tile([128, 128], BF16, tag="tr", bufs=2, name="tr_a")
                nc.tensor.transpose(
                    tr[:, :tl], xn_t[:tl, di * 128 : (di + 1) * 128],
                    identF[:tl, :tl],
                )
                # split the fp8 casts across DVE and ACT so neither engine
                # backs up while phase B's drains start to overlap phase A
                if di % 2 == 0:
                    nc.vector.tensor_copy(
                        xp[di // 2][:, di % 2, t0 : t0 + tl], tr[:, :tl]
                    )
                else:
                    nc.scalar.copy(
                        xp[di // 2][:, di % 2, t0 : t0 + tl], tr[:, :tl]
                    )

        # static per-channel vectors (emitted after phase A so the x-tile DMAs
        # lead the queue and the first transposes start sooner)
        b_ig = statics.tile([128, 2 * NKI], F32, tag="b_ig")
        nc.sync.dma_start(out=b_ig, in_=bias_ig.ap())
        b_bcd = statics.tile([128, 3 * NKI], F32, tag="b_bcd")
        nc.sync.dma_start(out=b_bcd, in_=bias_bcd.ap())
        a_t = statics.tile([128, NKI], F32, tag="a_t")
        nc.sync.dma_start(out=a_t, in_=a_in.ap())
        m0_t = statics.tile([128, 256], F32, tag="m0_t")
        nc.sync.dma_start(out=m0_t, in_=m0_in.ap())

        # ---- Phase B: u = (in @ xn + in_b) * sigmoid(gate @ xn + gate_b) ----
        # The u = u32*g multiplies are deferred by one iteration: when they
        # are emitted in program order their operands are already complete,
        # so the DVE/Pool queues never stall on a sigmoid while the next
        # iteration's psum drains sit behind them.
        def flush_mult(pmi, pu32, pgs):
            for bi, (n0, nl) in enumerate(TBLOCKS):
                eng = nc.vector if bi < 3 else nc.gpsimd
                eng.tensor_mul(
                    up[pmi // 2][:, pmi % 2, n0 : n0 + nl],
                    pu32[:, n0 : n0 + nl], pgs[bi][:, :nl],
                )

        pending = None
        for mi in range(NKI):
            w_s = wpool.tile([128, 2, KPD, 2, 128], FP8, tag="ws", name="w_ig_s")
            nc.sync.dma_start(out=w_s, in_=w_ig_r[:, mi])
            ps_us = [psA.tile([128, nl], F32, tag=("ps" if bi < 4 else "tr"),
                              bufs=(6 if bi < 4 else 2), name=f"ps_u{bi}")
                     for bi, (n0, nl) in enumerate(TBLOCKS)]
            for j in range(KPD):
                for bi, (n0, nl) in enumerate(TBLOCKS):
                    nc.tensor.matmul(
                        ps_us[bi], w_s[:, 0, j], xp[j][:, :, n0 : n0 + nl],
                        start=(j == 0), stop=(j == KPD - 1), perf_mode=DR,
                    )
            u32 = work.tile([128, T], F32, tag="fullT", name="u32")
            for bi, (n0, nl) in enumerate(TBLOCKS):
                # drains gate the gate-matmuls' psum slots; split DVE/ACT so
                # neither engine serializes the pipeline (gpsimd can't read
                # PSUM)
                if bi < 3:
                    nc.vector.tensor_scalar(
                        u32[:, n0 : n0 + nl], ps_us[bi],
                        S_U / (S_WI * S_X), b_ig[:, mi : mi + 1],
                        op0=OP.mult, op1=OP.add,
                    )
                else:
                    nc.scalar.activation(
                        u32[:, n0 : n0 + nl], ps_us[bi], AF.Identity,
                        bias=b_ig[:, mi : mi + 1], scale=S_U / (S_WI * S_X),
                    )
            ps_gs = [psA.tile([128, nl], F32, tag=("ps" if bi < 4 else "tr"),
                              bufs=(6 if bi < 4 else 2), name=f"ps_g{bi}")
                     for bi, (n0, nl) in enumerate(TBLOCKS)]
            for j in range(KPD):
                for bi, (n0, nl) in enumerate(TBLOCKS):
                    nc.tensor.matmul(
                        ps_gs[bi], w_s[:, 1, j], xp[j][:, :, n0 : n0 + nl],
                        start=(j == 0), stop=(j == KPD - 1), perf_mode=DR,
                    )
            g_sbs = []
            for bi, (n0, nl) in enumerate(TBLOCKS):
                g_sb = small.tile([128, 256], F32, tag="g_sb", bufs=10)
                nc.scalar.activation(
                    g_sb[:, :nl], ps_gs[bi], AF.Sigmoid,
                    bias=b_ig[:, NKI + mi : NKI + mi + 1],
                    scale=1.0 / (S_WG * S_X),
                )
                g_sbs.append(g_sb)
            if pending is not None:
                flush_mult(*pending)
            pending = (mi, u32, g_sbs)
        flush_mult(*pending)

        # ---- Phase C: b = b_w @ u + b_b ; h' = scan(a, b*BETA) ----
        for ji in range(NKI):
            w_s = wpool.tile([128, KPI, 2, 128], FP8, tag="ws", name="w_b_s")
            nc.sync.dma_start(out=w_s, in_=w_b_r[:, ji])
            pss = [psA.tile([128, nl], F32, tag=("ps" if bi < 4 else "tr"),
                            bufs=(6 if bi < 4 else 2), name=f"ps_b{bi}")
                   for bi, (n0, nl) in enumerate(TBLOCKS)]
            for j in range(KPI):
                for bi, (n0, nl) in enumerate(TBLOCKS):
                    nc.tensor.matmul(
                        pss[bi], w_s[:, j], up[j][:, :, n0 : n0 + nl],
                        start=(j == 0), stop=(j == KPI - 1), perf_mode=DR,
                    )
            b_full = work.tile([128, T], F32, tag="fullT", name="b_full")
            for bi, (n0, nl) in enumerate(TBLOCKS):
                nc.scalar.activation(
                    b_full[:, n0 : n0 + nl], pss[bi], AF.Identity,
                    bias=b_bcd[:, ji : ji + 1], scale=BETA / (S_WB * S_U),
                )
            a_bc = small.tile([128, 256], F32, tag="a_bc", bufs=2)
            nc.vector.memset(a_bc, 1.0)
            nc.vector.tensor_scalar_mul(a_bc, a_bc, a_t[:, ji : ji + 1])
            # block 0 carries the per-core reset mask (m0 column HALO)
            a_bc0 = small.tile([128, 256], F32, tag="a_bc0", bufs=2)
            nc.vector.tensor_scalar_mul(a_bc0, m0_t, a_t[:, ji : ji + 1])
            for bi, (n0, nl) in enumerate(TBLOCKS):
                init = 0.0 if bi == 0 else h[ji][:, n0 - 1 : n0]
                nc.vector.tensor_tensor_scan(
                    h[ji][:, n0 : n0 + nl],
                    (a_bc0 if bi == 0 else a_bc)[:, :nl],
                    b_full[:, n0 : n0 + nl], init, op0=OP.mult, op1=OP.add,
                )

        # ---- Phase D: h' *= (c_w @ u + c_b')   (in place) ----
        for ji in range(NKI):
            w_s = wpool.tile([128, KPI, 2, 128], FP8, tag="ws", name="w_c_s")
            nc.sync.dma_start(out=w_s, in_=w_c_r[:, ji])
            pss = [psA.tile([128, nl], F32, tag="ps", bufs=6, name=f"ps_c{bi}")
                   for bi, (n0, nl) in enumerate(OBLOCKS)]
            for j in range(KPI):
                for bi, (n0, nl) in enumerate(OBLOCKS):
                    nc.tensor.matmul(
                        pss[bi], w_s[:, j], up[j][:, :, n0 : n0 + nl],
                        start=(j == 0), stop=(j == KPI - 1), perf_mode=DR,
                    )
            for bi, (n0, nl) in enumerate(OBLOCKS):
                nc.vector.scalar_tensor_tensor(
                    h[ji][:, n0 : n0 + nl], pss[bi],
                    b_bcd[:, NKI + ji : NKI + ji + 1], h[ji][:, n0 : n0 + nl],
                    op0=OP.add, op1=OP.mult,
                )

        # phase-F statics, emitted late so the 2MB transfer rides under
        # phases D/E instead of competing with phase A's x loads
        w_om_t = statics.tile([128, KPI, 2, DIM], FP8, tag="w_om")
        nc.gpsimd.dma_start(out=w_om_t, in_=w_om_r)
        bias_ob_t = statics.tile([128, DIM], F32, tag="bias_ob")
        nc.gpsimd.dma_start(out=bias_ob_t, in_=bias_ob.ap())

        # ---- Phase E: y = h' + (d_w @ u + d_b)*S_Y  -> fp8 yp ----
        for ji in range(NKI):
            w_s = wpool.tile([128, KPI, 2, 128], FP8, tag="ws", name="w_d_s")
            nc.sync.dma_start(out=w_s, in_=w_d_r[:, ji])
            pss = [psA.tile([128, nl], F32, tag="ps", bufs=6, name=f"ps_d{bi}")
                   for bi, (n0, nl) in enumerate(OBLOCKS)]
            for j in range(KPI):
                for bi, (n0, nl) in enumerate(OBLOCKS):
                    nc.tensor.matmul(
                        pss[bi], w_s[:, j], up[j][:, :, n0 : n0 + nl],
                        start=(j == 0), stop=(j == KPI - 1), perf_mode=DR,
                    )
            dd32 = work.tile([128, T], F32, tag="fullT", name="dd32")
            for bi, (n0, nl) in enumerate(OBLOCKS):
                nc.scalar.activation(
                    dd32[:, n0 : n0 + nl], pss[bi], AF.Identity,
                    bias=b_bcd[:, 2 * NKI + ji : 2 * NKI + ji + 1],
                    scale=S_Y / (S_WD * S_U),
                )
            for bi, (n0, nl) in enumerate(OBLOCKS):
                nc.vector.tensor_add(
                    yp[ji // 2][:, ji % 2, n0 : n0 + nl],
                    h[ji][:, n0 : n0 + nl], dd32[:, n0 : n0 + nl],
                )

        # ---- Phase F: out[t, d] = y^T @ out_w + out_b + x  (token-major;
        # y is the matmul stationary so no transposes are needed, the bias
        # folds into the residual row, and each token tile streams its own
        # store as soon as it completes) ----
        for tt, (t0, tl) in enumerate(FTILES):
            x_r = statics.tile([128, DIM], BF16, tag=f"h{tt}",
                               name=f"x_row{tt}")
            (nc.gpsimd if tt % 2 else nc.sync).dma_start(
                out=x_r[:tl, :], in_=x_ap[t0 : t0 + tl, :])
            xb = statics.tile([128, DIM], F32, tag=f"h{8 + tt}", name=f"xb{tt}")
            nc.vector.tensor_add(xb, x_r, bias_ob_t)
            pss = [psA.tile([128, 256], F32, tag="ps", bufs=6, name=f"ps_o{bi}")
                   for bi in range(4)]
            for j in range(KPI):
                for bi in range(4):
                    nc.tensor.matmul(
                        pss[bi], yp[j][:, :, t0 : t0 + tl],
                        w_om_t[:, j, :, bi * 256 : (bi + 1) * 256],
                        start=(j == 0), stop=(j == KPI - 1), perf_mode=DR,
                    )
            for bi in range(4):
                nc.vector.scalar_tensor_tensor(
                    xb[:, bi * 256 : (bi + 1) * 256], pss[bi],
                    1.0 / (S_WO * S_Y), xb[:, bi * 256 : (bi + 1) * 256],
                    op0=OP.mult, op1=OP.add,
                )
            (nc.gpsimd if tt % 2 else nc.sync).dma_start(
                out=out_ap[t0 : t0 + tl, :], in_=xb[:tl, :])

    # walrus in this container only encodes 1 sync-wait on CTRL instructions
    from birfix_embed import patch_nc

    patch_nc(nc)
    return nc


# ---- embedded birfix (kernel.py must be self-contained) ----
def _enable_ldw_opt():
    """Flip walrus --enable-ldw-opt so consecutive same-weight matmuls skip
    the redundant LDWEIGHTS reload."""
    from concourse import bass_utils as _bu

    if getattr(_bu, "_ldw_opt_patched", False):
        return
    _orig = _bu.run_command

    def patched(argv, **kw):
        argv = ["--enable-ldw-opt=true" if a == "--enable-ldw-opt=false" else a
                for a in argv]
        return _orig(argv, **kw)

    _bu.run_command = patched
    _bu._ldw_opt_patched = True


# NOTE: not enabled — the Tile legalizer splits fp8 DoubleRow matmuls into
# explicit Ldweights+Matmult, and walrus rejects standalone Ldweights when
# --enable-ldw-opt=true. Ldweights dedup happens at legalize time instead.
# _enable_ldw_opt()


def _install_birfix():
    import json as _json
    import types

    mod = types.ModuleType("birfix_embed")

    CTRL = {"Drain", "NoOp", "EventSemaphore", "TriggeredCopy", "RegisterMove",
            "UnconditionalBranch", "Halt"}
    MAX_COMPUTE_WAITS = 1

    def dedup_ldweights(d):
        """Drop Ldweights whose stationary operand is already loaded.

        The Tile legalizer emits one Ldweights per (DoubleRow) Matmult; the
        PE array keeps its stationary across matmuls, so within a run of
        same-weight matmuls only the first load is needed. Any transpose or
        self-loading Matmult clobbers the array and resets tracking. The BIR
        here is post-schedule, so per-engine order is final."""
        removed = 0
        for fn in d.get("functions", []):
            for bb in fn.get("blocks", fn.get("basicblocks", [])):
                insts = bb.get("instructions", [])
                out = []
                loaded = None
                for inst in insts:
                    if inst.get("engine") != "PE":
                        out.append(inst)
                        continue
                    op = inst.get("opcode")
                    if op == "Ldweights":
                        sig = _json.dumps(
                            [inst.get("ins"), inst.get("perf_mode"),
                             inst.get("tile_position"), inst.get("tile_size"),
                             inst.get("is_transpose")],
                            sort_keys=True,
                        )
                        sync = inst.get("sync_info") or {}
                        if sig == loaded and not sync.get("on_update"):
                            waits = sync.get("on_wait") or []
                            if waits:
                                out.append({
                                    "engine": "PE", "ins": [],
                                    "name": inst["name"] + "_dd",
                                    "opcode": "NoOp", "outs": [],
                                    "sync_info": {"on_update": [],
                                                  "on_wait": waits},
                                })
                            removed += 1
                            continue
                        loaded = sig
                        out.append(inst)
                    elif op == "Matmult":
                        if inst.get("is_transpose") or inst.get("ldweights", True):
                            loaded = None
                        out.append(inst)
                    else:
                        out.append(inst)
                bb["instructions"] = out
        return removed

    def fix_bir_json(bir, max_ctrl=1, max_compute=MAX_COMPUTE_WAITS):
        d = _json.loads(bir)
        n_removed = dedup_ldweights(d)
        sys.stderr.write(f"birfix: removed {n_removed} redundant Ldweights\n")
        n_split = 0
        for fn in d.get("functions", []):
            for bb in fn.get("blocks", fn.get("basicblocks", [])):
                insts = bb.get("instructions", [])
                out = []
                changed = False
                for inst in insts:
                    sync = inst.get("sync_info")
                    cap = max_ctrl if inst.get("opcode") in CTRL else max_compute
                    if sync and len(sync.get("on_wait") or []) > cap:
                        waits = sync["on_wait"]
                        keep = waits[-cap:]
                        extra = waits[:-cap]
                        for i in range(0, len(extra), max_ctrl):
                            out.append(
                                {
                                    "engine": inst["engine"],
                                    "ins": [],
                                    "name": inst["name"] + f"_ws{i}",
                                    "opcode": "NoOp",
                                    "outs": [],
                                    "sync_info": {
                                        "on_update": [],
                                        "on_wait": extra[i : i + max_ctrl],
                                    },
                                }
                            )
                            n_split += 1
                        sync["on_wait"] = keep
                        changed = True
                    out.append(inst)
                if changed:
                    bb["instructions"] = out
        return _json.dumps(d).encode(), n_split

    def patch_nc(nc, max_ctrl=1, max_compute=MAX_COMPUTE_WAITS):
        orig = nc.to_json_bytes

        def patched():
            fixed, _ = fix_bir_json(orig(), max_ctrl, max_compute)
            return fixed

        nc.to_json_bytes = patched
        return nc

    mod.fix_bir_json = fix_bir_json
    mod.patch_nc = patch_nc
    sys.modules["birfix_embed"] = mod


_install_birfix()


def _install_ntff_hook():
    """The image lacks antenv.axon_hooks; recreate it so trace=True works."""
    import types

    if "antenv.axon_hooks" in sys.modules:
        return
    try:
        from trn_agent_boot.trn_boot import _ntff_profile_via_ctypes

        hook = _ntff_profile_via_ctypes("/opt/axon/libaxon_pjrt.so")
    except Exception:
        hook = None
    mod = types.ModuleType("antenv.axon_hooks")
    mod.get_axon_ntff_profile_hook = lambda: hook
    mod.set_axon_ntff_profile_hook = lambda h: None
    sys.modules["antenv.axon_hooks"] = mod


# ---- two-pass build: capture schedule manifest, reorder matmuls to
# weight-major (dependency- and slot-safe), rebuild with the manifest ----
def _fishpath_compat():
    from concourse._compat import FishPath

    if not hasattr(FishPath, "open"):
        def _open(self, mode="r"):
            if "w" in mode:
                self._path.parent.mkdir(parents=True, exist_ok=True)
            return open(self._path, mode)
        FishPath.open = _open
    if not hasattr(FishPath, "makedirs"):
        FishPath.makedirs = (
            lambda self: self._path.mkdir(parents=True, exist_ok=True))
    if not hasattr(FishPath, "is_file"):
        FishPath.is_file = lambda self: self._path.is_file()
    if not hasattr(FishPath, "parent"):
        FishPath.parent = property(lambda self: FishPath(self._path.parent))
    if not hasattr(FishPath, "__fspath__"):
        FishPath.__fspath__ = lambda self: str(self._path)


def _rewrite_manifest(mdir, bir, releases):
    """Reorder the captured manifest so DoubleRow matmuls run weight-major
    (j-outer), respecting data deps and tile slot reuse, so the birfix
    Ldweights dedup can drop redundant PE weight loads."""
    import glob as _glob
    import heapq
    import json as _json
    from collections import defaultdict

    mpath = _glob.glob(os.path.join(mdir, "*.json"))[0]
    with open(mpath) as f:
        m = _json.load(f)
    dpath = _glob.glob(os.path.join(mdir, "*_debug_info/instruction_deps.json"))[0]
    with open(dpath) as f:
        deps = _json.load(f)

    meta = {}
    readers = defaultdict(set)
    writers = defaultdict(set)
    for fn in bir.get("functions", []):
        for bb in fn.get("blocks", fn.get("basicblocks", [])):
            for inst in bb.get("instructions", []):
                nm = inst.get("name")
                for a in inst.get("ins", []) or []:
                    if isinstance(a, dict) and a.get("memref"):
                        readers[a["memref"]].add(nm)
                for a in inst.get("outs", []) or []:
                    if isinstance(a, dict) and a.get("memref"):
                        writers[a["memref"]].add(nm)
                if (inst.get("opcode") == "Matmult"
                        and not inst.get("is_transpose")
                        and inst.get("perf_mode") == "DoubleRow"):
                    wap = inst["ins"][1]
                    meta[nm] = (wap["memref"], wap["offset"])
    for relname, tname in releases.items():
        readers[tname].add(relname)

    slot_groups = defaultdict(list)
    for tname, (addr, space) in m["addresses"].items():
        slot_groups[(space, addr)].append(tname)

    def alloc_id(tname):
        try:
            return int(tname.rsplit("_", 1)[1])
        except ValueError:
            return 0

    for block, order in m["order"].items():
        rank = {}
        groups = defaultdict(list)
        for pos, e in enumerate(order):
            rank[e["name"]] = pos
            if e["engine"] == "PE" and e["name"] in meta:
                groups[meta[e["name"]][0]].append(pos)
        for w, positions in groups.items():
            tagged = sorted(
                (meta[order[p]["name"]][1], i, order[p]["name"])
                for i, p in enumerate(positions)
            )
            for p, (_, _, name) in zip(positions, tagged):
                rank[name] = p
        # emit all non-PE entries (drains, DMAs, virtual releases) as soon as
        # they are ready — their original positions are stale after the
        # matmul permutation and would otherwise delay psum-slot releases,
        # breaking the same-weight matmul runs the dedup relies on
        for e in order:
            if e["engine"] != "PE":
                rank[e["name"]] = -1
        for relname in releases:
            if relname in rank:
                rank[relname] = -1

        entry_by_name = {e["name"]: e for e in order}
        succ = defaultdict(list)
        indeg = {e["name"]: 0 for e in order}
        edges = set()

        def add_edge(a, b):
            if a != b and (a, b) not in edges:
                edges.add((a, b))
                succ[a].append(b)
                indeg[b] += 1

        for name, dd in deps.items():
            if name not in indeg:
                continue
            for pred in set(dd.get("pre_data", []) + dd.get("pre_no_sync", [])):
                if pred in indeg:
                    add_edge(pred, name)
        for (space, addr), tiles in slot_groups.items():
            if len(tiles) < 2:
                continue
            tiles = sorted(tiles, key=alloc_id)
            for t1, t2 in zip(tiles, tiles[1:]):
                uses = (readers[t1] | writers[t1]) & indeg.keys()
                wrts = writers[t2] & indeg.keys()
                for u in uses:
                    for wv in wrts:
                        add_edge(u, wv)
        # chain weight-groups so a group's matmuls finish before the next
        # group starts — keeps same-weights matmuls consecutive on the PE
        # queue (the Ldweights dedup then drops ~4/5 of the weight loads)
        glist = sorted(groups.items(), key=lambda kv: min(kv[1]))
        for (w1, p1), (w2, p2) in zip(glist, glist[1:]):
            last = max(p1, key=lambda p: rank[order[p]["name"]])
            first = min(p2, key=lambda p: rank[order[p]["name"]])
            add_edge(order[last]["name"], order[first]["name"])

        heap = [(rank[nm], nm) for nm, c in indeg.items() if c == 0]
        heapq.heapify(heap)
        new_order = []
        while heap:
            _, nm = heapq.heappop(heap)
            new_order.append(entry_by_name[nm])
            for s in succ[nm]:
                indeg[s] -= 1
                if indeg[s] == 0:
                    heapq.heappush(heap, (rank[s], s))
        assert len(new_order) == len(order), (len(new_order), len(order))
        m["order"][block] = new_order

    with open(mpath, "w") as f:
        _json.dump(m, f)


def build_nc_manifest():
    import json as _json
    import shutil
    import tempfile

    _fishpath_compat()
    mdir = tempfile.mkdtemp(prefix="bass_manifest_")
    saved = {k: os.environ.get(k) for k in
             ("TILE_CAPTURE_MANIFEST_PATH", "TILE_SCHEDULER",
              "TILE_LOAD_MANIFEST_PATH")}
    try:
        os.environ["TILE_CAPTURE_MANIFEST_PATH"] = mdir
        os.environ.pop("TILE_SCHEDULER", None)
        os.environ.pop("TILE_LOAD_MANIFEST_PATH", None)
        nc1 = build_nc()
        bir = _json.loads(nc1.to_json_bytes())
        releases = {}
        for nm, inst in nc1.inst_map.items():
            if (type(inst).__name__ == "BassTileRelease"
                    and inst.bass_tile is not None):
                releases[nm] = inst.bass_tile.name
        _rewrite_manifest(mdir, bir, releases)
        del nc1, bir
        os.environ.pop("TILE_CAPTURE_MANIFEST_PATH", None)
        os.environ["TILE_SCHEDULER"] = "manifest"
        os.environ["TILE_LOAD_MANIFEST_PATH"] = mdir
        nc2 = build_nc()
        return nc2
    finally:
        for k, v in saved.items():
            if v is None:
                os.environ.pop(k, None)
            else:
                os.environ[k] = v
        shutil.rmtree(mdir, ignore_errors=True)


def _prep_dr(W, s):
    """[M, K] weight -> DoubleRow strip layout [128, MT*KP*2*128] fp8,
    where strip[p, mt, j, i, m] = (W*s)[mt*128+m, j*256+i*128+p]."""
    M, K = W.shape
    MT, KP = M // 128, K // 256
    Wq = np.clip(W.astype(np.float64) * s, -240.0, 240.0)
    arr = np.ascontiguousarray(Wq.T).reshape(KP, 2, 128, MT, 128)
    arr = np.ascontiguousarray(arr.transpose(2, 3, 0, 1, 4))  # p mt j i m
    return arr.reshape(128, MT * KP * 2 * 128).astype(E4M3)


def _prep_mov(W, s):
    """[M, K] weight -> DoubleRow moving layout [128, KP*2*M] fp8,
    where mov[p, j, i, d] = (W*s)[d, j*256+i*128+p]."""
    M, K = W.shape
    KP = K // 256
    Wq = np.clip(W.astype(np.float64) * s, -240.0, 240.0)
    arr = np.ascontiguousarray(Wq.T).reshape(KP, 2, 128, M)
    arr = np.ascontiguousarray(arr.transpose(2, 0, 1, 3))  # p j i d
    return arr.reshape(128, KP * 2 * M).astype(E4M3)


def _prep_shared(norm_w, in_w, in_b, gate_w, gate_b, b_w, b_b, c_w, c_b, d_w, d_b,
                 out_w, out_b, a_log):
    c = np.ascontiguousarray
    f = np.float32
    a = np.exp(-np.logaddexp(0.0, a_log.astype(np.float64))).astype(f)
    in_s = _prep_dr(in_w * norm_w[None, :], S_WI)  # [128, 16*1024]
    gate_s = _prep_dr(gate_w * norm_w[None, :], S_WG)
    ig = np.stack(
        [in_s.reshape(128, NKI, KPD * 2 * 128),
         gate_s.reshape(128, NKI, KPD * 2 * 128)], axis=2
    )  # [128, mi, half, ...]
    shared = {
        "w_ig": c(ig.reshape(128, NKI * 2 * KPD * 2 * 128)),
        "w_b": _prep_dr(b_w, S_WB),
        "w_c": _prep_dr(c_w, S_WC),
        "w_d": _prep_dr(d_w, S_WD),
        "w_om": _prep_mov(out_w, S_WO),
        "bias_ig": c(np.concatenate([in_b * S_U, gate_b]).astype(f)
                     .reshape(2 * NKI, 128).T),
        "bias_bcd": c(np.concatenate(
            [b_b * BETA, c_b * (S_WC * S_U), d_b * S_Y]
        ).astype(f).reshape(3 * NKI, 128).T),
        "bias_ob": c(np.broadcast_to(out_b.astype(f), (128, DIM)).copy()),
        "a_in": c(a.reshape(NKI, 128).T),
    }
    return shared


def kernel(x, norm_w, in_w, in_b, gate_w, gate_b, b_w, b_b, c_w, c_b, d_w, d_b,
           out_w, out_b, a_log, _trace=False):
    # inputs may be jax arrays; convert up front so host math stays in numpy
    x, norm_w, in_w, in_b, gate_w, gate_b = (
        np.asarray(v, np.float32) for v in (x, norm_w, in_w, in_b, gate_w, gate_b))
    b_w, b_b, c_w, c_b, d_w, d_b, out_w, out_b, a_log = (
        np.asarray(v, np.float32)
        for v in (b_w, b_b, c_w, c_b, d_w, d_b, out_w, out_b, a_log))

    if "nc" not in _CACHED:
        # plain build: measured on HW, LDWEIGHTS pipelines under matmuls,
        # so the manifest-reordered (weight-major) schedule buys nothing
        # and its serialization costs ~40us; keep build_nc_manifest around
        # for experiments
        _CACHED["nc"] = build_nc()
    nc = _CACHED["nc"]

    shared = _prep_shared(norm_w, in_w, in_b, gate_w, gate_b, b_w, b_b, c_w, c_b,
                          d_w, d_b, out_w, out_b, a_log)
    m0_ones = np.ones((128, 256), np.float32)
    m0_reset = m0_ones.copy()
    m0_reset[:, HALO] = 0.0  # kills the recurrence carry at the true seq start
    in_maps = []
    for core in range(8):
        bi, sh = core // 2, core % 2
        m = dict(shared)
        if sh == 0:
            sl = np.concatenate(
                [np.zeros((HALO, DIM), np.float32), x[bi, 0 : S // 2]], axis=0)
            m["m0"] = m0_reset
        else:
            sl = x[bi, S // 2 - HALO : S]
            m["m0"] = m0_ones
        m["x"] = np.ascontiguousarray(sl.astype(ml_dtypes.bfloat16))
        in_maps.append(m)

    kw = {}
    if _trace:
        _install_ntff_hook()
        kw = dict(trace=True, trace_cores=[0], trace_events=False)
    res = run_bass_kernel_spmd(nc, in_maps, core_ids=list(range(8)), **kw)
    _CACHED["last_result"] = res

    outp = np.empty((B, S, DIM), np.float32)
    for core in range(8):
        bi, sh = core // 2, core % 2
        o = res.results[core]["out"]
        outp[bi, sh * (S // 2) : (sh + 1) * (S // 2)] = o[HALO : HALO + S // 2]
    return outp
